# revision 36
# baseline (speedup 1.0000x reference)
"""AnalyticalPointNetLK forward on 8 Trainium2 NeuronCores.

Pure data parallel: batch element b -> core b. Everything (PointNet forward,
argmax, analytical Jacobian, 6x6 inverse, Gauss-Newton iterations, se(3) exp
map) runs on-device in one NEFF.

Self-contained: only needs the container's concourse stack.
"""
import sys
import types

import numpy as np


# ---------------------------------------------------------------- env setup
def _setup_env():
    try:
        import concourse.bass  # noqa: F401
    except ImportError:
        sys.path.insert(0, "/opt/trn_rl_repo")

    # Inject antenv.axon_hooks (missing in this image) so trace=True works.
    try:
        import antenv

        if not hasattr(antenv, "axon_hooks"):
            _m = types.ModuleType("antenv.axon_hooks")
            _m._hook = None
            _m.set_axon_ntff_profile_hook = lambda h: setattr(_m, "_hook", h)
            _m.get_axon_ntff_profile_hook = lambda: _m._hook
            sys.modules["antenv.axon_hooks"] = _m
            antenv.axon_hooks = _m
            try:
                from trn_agent_boot.trn_boot import _ntff_profile_via_ctypes

                h = _ntff_profile_via_ctypes("/opt/axon/libaxon_pjrt.so")
                if h is not None:
                    _m.set_axon_ntff_profile_hook(h)
            except Exception:
                pass
    except ImportError:
        pass

    # Split multi-wait exit Drain (this walrus rejects >1 sem wait on CTRL).
    from concourse.tile import TileContext
    from concourse.vector_clock import ScopedClock

    if not getattr(TileContext, "_drain_split_installed", False):

        def _patched(self, tick_clock, wait_clock):
            nc = self.nc
            drain_inst = nc.sync.drain()
            wait_clock.add_sem_waits(
                drain_inst.ins, ScopedClock({None: tick_clock.global_clock})
            )
            si = drain_inst.ins.sync_info
            if si is not None and si.on_wait and len(si.on_wait) > 1:
                waits = list(si.on_wait)
                si.on_wait = waits[:1]
                for w in waits[1:]:
                    extra = nc.sync.drain()
                    esi = extra.ins.sync_info
                    if esi is None:
                        import bass_rust

                        extra.ins.sync_info = bass_rust.SyncInfo(
                            on_wait=[w], on_update=[]
                        )
                    else:
                        esi.on_wait = [w]
            nc.all_engine_barrier()
            assert self.sems is not None
            popped = nc._tile_sem_poison_stack.pop()
            assert popped is self._sem_poison
            nc.clear_and_free_semaphores(list(self.sems.allocated().values()))
            nc.all_engine_barrier()

        TileContext._drain_and_barrier = _patched
        TileContext._drain_split_installed = True


_setup_env()

import concourse.bass as bass  # noqa: E402
import concourse.mybir as mybir  # noqa: E402
from concourse.tile import TileContext  # noqa: E402
from concourse.bass_utils import run_bass_kernel_spmd  # noqa: E402

F32 = mybir.dt.float32
F32R = mybir.dt.float32r
ALU = mybir.AluOpType
ACTF = mybir.ActivationFunctionType
AX = mybir.AxisListType

N, K, C1, C2 = 1024, 1024, 64, 128
NCH = 8
EPS = 1e-5
NEG_INF = -3.0e38

# exp-map series (Horner, highest degree first): s1=sin t/t, s2=(1-cos t)/t^2,
# s3=(t-sin t)/t^3 as series in t2=t^2, up to t^12
SER = np.array(
    [
        [1 / 6227020800, 1 / 87178291200, 1 / 1307674368000],
        [-1 / 39916800, -1 / 479001600, -1 / 6227020800],
        [1 / 362880, 1 / 3628800, 1 / 39916800],
        [-1 / 5040, -1 / 40320, -1 / 362880],
        [1 / 120, 1 / 720, 1 / 5040],
        [-1 / 6, -1 / 24, -1 / 120],
        [1.0, 0.5, 1 / 6],
    ],
    np.float32,
)

# ------------------------------------------------------------- consts blob
OFF_IDENT = 0
OFF_IOTA = OFF_IDENT + 128 * 128
OFF_SER = OFF_IOTA + 1024
OFF_S = OFF_SER + 21          # S0|S1|S2 skew generators, [3 rows x 9]
OFF_I3 = OFF_S + 27           # [3 x 3]
OFF_I34 = OFF_I3 + 9          # [3 x 4]
OFF_E4 = OFF_I34 + 12         # [1 x 4]
OFF_I4 = OFF_E4 + 4           # [4 x 4]
OFF_E16 = OFF_I4 + 16         # [16 x 3]
OFF_ONES = OFF_E16 + 48       # [6 x 8] ones
OFF_E6 = OFF_ONES + 48        # [6 x 36]: cE6[:, 6i:6i+6] = row-i-ones matrix
OFF_I6 = OFF_E6 + 216         # [6 x 6]
OFF_I9ROW = OFF_I6 + 36       # [1 x 9] I3 row-major
OFF_S9ROW = OFF_I9ROW + 9     # [1 x 27] skew generators row-major
OFF_ONE1K = OFF_S9ROW + 27    # [1024] ones
OFF_ZERO1K = OFF_ONE1K + 1024  # [1024] zeros
CONST_LEN = OFF_ZERO1K + 1024


def _build_consts():
    c = np.zeros(CONST_LEN, np.float32)
    c[OFF_IDENT:OFF_IDENT + 128 * 128] = np.eye(128, dtype=np.float32).ravel()
    c[OFF_IOTA:OFF_IOTA + 1024] = np.arange(1024, dtype=np.float32)
    c[OFF_SER:OFF_SER + 21] = SER.ravel()
    S0 = np.array([[0, 0, 0], [0, 0, -1], [0, 1, 0]], np.float32)
    S1 = np.array([[0, 0, 1], [0, 0, 0], [-1, 0, 0]], np.float32)
    S2 = np.array([[0, -1, 0], [1, 0, 0], [0, 0, 0]], np.float32)
    c[OFF_S:OFF_S + 27] = np.concatenate([S0, S1, S2], axis=1).ravel()
    c[OFF_I3:OFF_I3 + 9] = np.eye(3, dtype=np.float32).ravel()
    c[OFF_I34:OFF_I34 + 12] = np.eye(3, 4, dtype=np.float32).ravel()
    c[OFF_E4:OFF_E4 + 4] = np.array([0, 0, 0, 1], np.float32)
    c[OFF_I4:OFF_I4 + 16] = np.eye(4, dtype=np.float32).ravel()
    c[OFF_E16:OFF_E16 + 48] = np.eye(16, 3, dtype=np.float32).ravel()
    c[OFF_ONES:OFF_ONES + 48] = 1.0
    e6 = np.zeros((6, 36), np.float32)
    for i in range(6):
        e6[i, 6 * i:6 * i + 6] = 1.0
    c[OFF_E6:OFF_E6 + 216] = e6.ravel()
    c[OFF_I6:OFF_I6 + 36] = np.eye(6, dtype=np.float32).ravel()
    c[OFF_I9ROW:OFF_I9ROW + 9] = np.eye(3, dtype=np.float32).ravel()
    c[OFF_S9ROW:OFF_S9ROW + 27] = np.concatenate(
        [np.array([[0, 0, 0], [0, 0, -1], [0, 1, 0]], np.float32).ravel(),
         np.array([[0, 0, 1], [0, 0, 0], [-1, 0, 0]], np.float32).ravel(),
         np.array([[0, -1, 0], [1, 0, 0], [0, 0, 0]], np.float32).ravel()])
    c[OFF_ONE1K:OFF_ONE1K + 1024] = 1.0
    return c


CONSTS = _build_consts()


def _split_multi_waits(nc):
    """walrus (this build) accepts at most one sem wait per instruction on
    several opcode classes. Split any instruction with >1 waits by inserting
    same-engine nops, each carrying one wait, immediately before it."""
    import bass_rust

    def _make_nop(engine):
        h = nc.engines[engine]
        inst = h.nop(nofuse=True)
        # nop() appended to the current bb; detach it
        for f in nc.m.functions:
            for bb in f.blocks:
                lst = bb.instructions
                if lst and lst[-1] is inst.ins:
                    lst.pop()
                    return inst.ins
        raise RuntimeError("nop not found for detach")

    for f in nc.m.functions:
        for bb in f.blocks:
            lst = bb.instructions
            out = []
            changed = False
            for inst in list(lst):
                si = inst.sync_info
                if si is not None and si.on_wait and len(si.on_wait) > 1:
                    waits = list(si.on_wait)
                    for w in waits[:-1]:
                        nop = _make_nop(inst.engine)
                        nop.sync_info = bass_rust.SyncInfo(
                            on_wait=[w], on_update=[])
                        out.append(nop)
                    si.on_wait = [waits[-1]]
                    changed = True
                out.append(inst)
            if changed:
                lst.clear()
                lst.extend(out)


def _bcast_free(ap, count):
    """Append a stride-0 free dim (broadcast) to an AP view."""
    dims = [list(d) for d in ap.ap] + [[0, count]]
    return bass.AP(tensor=ap.tensor, offset=ap.offset, ap=dims)


def build_kernel(maxiter: int, use_f32r: bool = False, reduce_ttr: bool = True,
                 debug: bool = False):
    nc = bass.Bass("TRN2", target_bir_lowering=False, debug=False,
                   num_devices=8)

    def din(name, shape, dtype=F32):
        return nc.dram_tensor(name, shape, dtype, kind="ExternalInput").ap()

    p0d = din("p0", [N, 3])
    p1d = din("p1", [N, 3])
    W1d = din("W1", [C1, 3])
    W2Td = din("W2T", [C1, C2])
    W3Td = din("W3T", [C2, K])
    W3nd = din("W3", [K, C2])
    prmd = din("prm", [128, 50])
    constsd = din("consts", [CONST_LEN])
    outd = nc.dram_tensor("out", [K], F32, kind="ExternalOutput").ap()

    with TileContext(nc) as tc:
        with (
            tc.tile_pool(name="per", bufs=1) as per,
            tc.tile_pool(name="pbig", bufs=2, space="PSUM") as pbig,
            tc.tile_pool(name="psm", bufs=2, space="PSUM") as psm,
            tc.tile_pool(name="pfill", bufs=1, space="PSUM") as pfill,
            tc.tile_pool(name="bbp", bufs=2) as bbp,
            tc.tile_pool(name="dramp", bufs=1, space="DRAM") as dramp,
        ):
            def T(shape, tag, dtype=F32):
                return per.tile(shape, dtype, tag=tag, name=tag)

            def big_psum():
                return pbig.tile([128, 1024], F32, tag="big", name="bigp")

            def small_psum():
                return psm.tile([128, 64], F32, tag="small", name="smallp")

            def mmr(out, lhsT, rhs, fast=None, **kw):
                f = use_f32r if fast is None else (fast and use_f32r)
                if f:
                    nc.tensor.matmul(out, lhsT.bitcast(F32R),
                                     rhs.bitcast(F32R), **kw)
                else:
                    nc.tensor.matmul(out, lhsT, rhs, **kw)

            def rr(ap):
                # producers feeding an fp32r matmul must round to fp32r
                return ap.bitcast(F32R) if use_f32r else ap

            def cfill(dst, ones: bool, f32r: bool = False):
                # memset replacement (this walrus rejects InstMemset):
                # broadcast-DMA a constant vector from the consts blob
                p, fsz = dst.shape[0], 1
                for d in dst.shape[1:]:
                    fsz *= d
                assert fsz <= 1024
                off = OFF_ONE1K if ones else OFF_ZERO1K
                srcap = bass.AP(tensor=constsd.tensor,
                                offset=constsd.offset + off,
                                ap=[[0, p], [1, fsz]])
                if f32r and use_f32r:
                    srcap = srcap.bitcast(F32R)
                    dst = rr(dst)
                nc.sync.dma_start(
                    out=dst.rearrange(
                        " ".join(f"d{i}" for i in range(len(dst.shape)))
                        + " -> d0 ("
                        + " ".join(f"d{i}" for i in range(1, len(dst.shape)))
                        + ")") if len(dst.shape) > 2 else dst,
                    in_=srcap)

            # ---------------- const + input DMAs
            def cdma(shape, tag, off, length):
                t = T(shape, tag)
                nc.sync.dma_start(
                    out=t,
                    in_=constsd[off:off + length].rearrange(
                        "(p f) -> p f", p=shape[0]))
                return t

            ident = cdma([128, 128], "ident", OFF_IDENT, 128 * 128)
            ciota = cdma([1, 1024], "ciota", OFF_IOTA, 1024)
            cser = cdma([1, 21], "cser", OFF_SER, 21)
            cS = cdma([3, 9], "cS", OFF_S, 27)
            cI3 = cdma([3, 3], "cI3", OFF_I3, 9)
            cI34 = cdma([3, 4], "cI34", OFF_I34, 12)
            cE4 = cdma([1, 4], "cE4", OFF_E4, 4)
            cI4 = cdma([4, 4], "cI4", OFF_I4, 16)
            cones = cdma([6, 8], "cones", OFF_ONES, 48)
            cE6 = cdma([6, 36], "cE6", OFF_E6, 216)
            cI6 = cdma([6, 6], "cI6", OFF_I6, 36)
            cI9r = cdma([1, 9], "cI9r", OFF_I9ROW, 9)
            cS9r = cdma([1, 27], "cS9r", OFF_S9ROW, 27)

            prm = T([128, 50], "prm")
            nc.sync.dma_start(out=prm, in_=prmd[:, :])
            W1sb = T([C1, 3], "W1sb")
            nc.sync.dma_start(out=W1sb, in_=W1d[:, :])
            W2T = T([C1, C2], "W2T")
            nc.sync.dma_start(out=rr(W2T), in_=W2Td[:, :].bitcast(F32R)
                              if use_f32r else W2Td[:, :])
            p0c4 = T([128, 32], "p0c")
            cfill(p0c4, ones=True)
            nc.sync.dma_start(
                out=p0c4.rearrange("p (k d) -> p k d", d=4)[:, :, 0:3],
                in_=p0d.rearrange("(p k) d -> p k d", p=128))
            p1c = T([128, 32], "p1c")
            cfill(p1c, ones=True)
            nc.sync.dma_start(
                out=p1c.rearrange("p (k d) -> p k d", d=4)[:, :, 0:3],
                in_=p1d.rearrange("(p k) d -> p k d", p=128))
            W3T = T([C2, K], "W3T")
            nc.sync.dma_start(out=rr(W3T), in_=W3Td[:, :].bitcast(F32R)
                              if use_f32r else W3Td[:, :])
            W3n = T([128, 1024], "W3n")
            w3n_src = bass.AP(tensor=W3nd.tensor, offset=W3nd.offset,
                              ap=[[128, 128], [16384, 8], [1, 128]])
            nc.sync.dma_start(
                out=W3n.rearrange("p (c d) -> p c d", d=128), in_=w3n_src)
            ones128 = T([128, 1], "ones128")
            cfill(ones128, ones=True)

            # ---------------- param prep: a = gamma*rsqrt(rv+eps),
            # bb = a*(b-rm)+beta  (rsqrt: exact sqrt + bit-exact reciprocal)
            def bn_fold(gam, bet, rm, rv, bias, pshape, tagp):
                t = T(pshape, tagp + "_t")
                nc.vector.tensor_scalar_add(t, rv, EPS)
                s = T(pshape, tagp + "_s")
                nc.scalar.sqrt(s, t)
                y = T(pshape, tagp + "_y")
                nc.vector.reciprocal(y, s)
                a = T(pshape, tagp + "_a")
                nc.vector.tensor_mul(a, gam, y)
                bb = T(pshape, tagp + "_bb")
                nc.vector.tensor_sub(bb, bias, rm)
                nc.vector.tensor_mul(bb, bb, a)
                nc.vector.tensor_add(bb, bb, bet)
                return a, bb

            ablk, bbblk = bn_fold(prm[:, 0:10], prm[:, 10:20],
                                  prm[:, 20:30], prm[:, 30:40],
                                  prm[:, 40:50], [128, 10], "bn")
            a1, bb1 = ablk[0:C1, 0:1], bbblk[0:C1, 0:1]
            a2, bb2 = ablk[0:C2, 1:2], bbblk[0:C2, 1:2]
            a3, bb3 = ablk[:, 2:10], bbblk[:, 2:10]

            # ---------------- weight prep
            W1s = T([C1, 3], "W1s")
            nc.vector.tensor_scalar_mul(W1s, W1sb, a1)
            W1s4 = T([C1, 4], "W1s4")
            nc.vector.tensor_copy(W1s4[:, 0:3], W1s)
            nc.vector.tensor_copy(W1s4[:, 3:4], bb1)
            W1aug = T([4, C1], "W1aug")
            tp = small_psum()
            nc.tensor.transpose(tp[0:4, 0:C1], W1s4, ident[0:C1, 0:C1])
            nc.scalar.copy(rr(W1aug), tp[0:4, 0:C1])

            W2aug = T([C1 + 1, C2], "W2aug")
            a2dram = dramp.tile([C2], F32, tag="a2d", name="a2d")
            nc.sync.dma_start(out=a2dram, in_=a2)
            a2bc = T([C1, C2], "a2bc")
            a2bc_src = bass.AP(tensor=a2dram.tensor, offset=a2dram.offset,
                               ap=[[0, C1], [1, C2]])
            nc.sync.dma_start(out=a2bc, in_=a2bc_src)
            nc.vector.tensor_mul(rr(W2aug[0:C1, :]), W2T, a2bc)
            tp4 = psm.tile([128, 128], F32, tag="small", name="smallp")
            nc.tensor.transpose(tp4[0:1, 0:C2], bb2, ident[0:C2, 0:C2])
            nc.scalar.copy(rr(W2aug[C1:C1 + 1, :]), tp4[0:1, 0:C2])

            # ---------------- p0 / p1 transposed homogeneous
            p0hex = T([16, N], "p0hex")
            p1hex = T([4, N], "p1hex")
            cfill(p0hex, ones=False, f32r=True)
            for (srct, dsttile) in ((p0c4, p0hex), (p1c, p1hex)):
                tpp = big_psum()
                for c in range(8):
                    nc.tensor.transpose(
                        tpp[0:4, c * 128:(c + 1) * 128],
                        srct[:, c * 4:(c + 1) * 4], ident)
                nc.vector.tensor_copy(rr(dsttile[0:4, :]), tpp[0:4, 0:N])

            iotabc = T([128, 1024], "iotabc")
            iot_src = bass.AP(tensor=constsd.tensor,
                              offset=constsd.offset + OFF_IOTA,
                              ap=[[0, 128], [1, 1024]])
            nc.sync.dma_start(out=iotabc, in_=iot_src)

            # ---------------- shared tiles
            scratch = T([128, 1024], "scratch")
            x1s = T([C1 + 1, N], "x1s")
            cfill(x1s[C1:C1 + 1, :], ones=True)
            x2s = T([C2, N], "x2s")

            # PE-warmth fillers: junk fp32 matmuls on constant tiles, issued
            # into per-iteration PE idle windows so the tensor clock stays
            # at its top p-state (cold restarts run ~1.8x slower)
            def pe_fill(n):
                for _ in range(n):
                    fj = pfill.tile([128, 512], F32, tag="fill",
                                    name="fillp")
                    nc.tensor.matmul(fj, W3T[:, 0:128], W3n[:, 0:512])

            def fwd12(lhsT1, phex, fast=True):
                u1 = big_psum()
                nc.tensor.matmul(u1[0:C1, 0:512], lhsT1, phex[0:4, 0:512])
                nc.tensor.matmul(u1[0:C1, 512:1024], lhsT1,
                                 phex[0:4, 512:1024])
                nc.scalar.activation(x1s[0:C1, 0:512], u1[0:C1, 0:512],
                                     ACTF.Relu)
                nc.scalar.activation(x1s[0:C1, 512:1024],
                                     u1[0:C1, 512:1024], ACTF.Relu)
                u2 = big_psum()
                nc.tensor.matmul(u2[0:C2, 0:512], W2aug, x1s[:, 0:512])
                nc.tensor.matmul(u2[0:C2, 512:1024], W2aug,
                                 x1s[:, 512:1024])
                nc.scalar.activation(x2s[:, 0:512], u2[0:C2, 0:512],
                                     ACTF.Relu)
                nc.scalar.activation(x2s[:, 512:1024],
                                     u2[0:C2, 512:1024], ACTF.Relu)

            def l3_chunks(m_out, idx_out, fast=True, after_chunk=None):
                for c in range(NCH):
                    ps = big_psum()
                    lh = W3T[:, c * 128:(c + 1) * 128]
                    nc.tensor.matmul(ps[0:C2, 0:512], lh, x2s[:, 0:512])
                    nc.tensor.matmul(ps[0:C2, 512:1024], lh,
                                     x2s[:, 512:1024])
                    nc.vector.tensor_reduce(
                        m_out[:, c:c + 1], ps[:, 0:1024], axis=AX.X,
                        op=ALU.max)
                    if idx_out is not None:
                        nc.vector.scalar_tensor_tensor(
                            out=scratch, in0=ps[:, 0:1024],
                            scalar=m_out[:, c:c + 1], in1=iotabc,
                            op0=ALU.is_ge, op1=ALU.mult,
                            accum_out=idx_out[:, c:c + 1])
                    if after_chunk is not None:
                        after_chunk(c)

            # ---------------- setup forward on p0 (masks + argmax)
            fwd12(W1aug, p0hex, fast=False)

            # masks via Act sign (x>=0 post-relu, so sign == is_gt-0 mask);
            # keeps the DVE free for the argmax phase
            m1f = T([C1, N], "m1f")
            nc.scalar.sign(m1f, x1s[0:C1, :])
            m2f = T([C2, N], "m2f")
            nc.scalar.sign(m2f, x2s)

            # ---------------- gather-table build (idx-independent, emitted
            # before the argmax L3 so its PE/DMA work overlaps DVE reduces)
            import concourse.tile as _tile_mod

            # m2f transposed: m2fT[:, 128*nu:...] = m2f chunk nu ^T
            m2fT = T([128, 1024], "m2fT")
            for nu in range(8):
                tpm = psm.tile([128, 128], F32, tag="small", name="smallp")
                nc.tensor.transpose(tpm, m2f[:, 128 * nu:128 * (nu + 1)],
                                    ident)
                nc.scalar.copy(m2fT[:, 128 * nu:128 * (nu + 1)], tpm)

            t1m = []
            for i in range(3):
                t = T([C1, N], f"t1m{i}")
                nc.scalar.mul(rr(t), m1f, W1s[:, i:i + 1])
                t1m.append(t)

            CATW = 388
            tcat = T([128, 8 * CATW], "tcat")
            tcat3 = tcat.rearrange("p (nu w) -> p nu w", w=CATW)
            # t2mT chunks: psum[128n, 512] holds nu-batch of 4 for one i
            for i in range(3):
                for half in range(2):
                    psb = pbig.tile([128, 512], F32, tag="big", name="ttp")
                    for q in range(4):
                        nu = half * 4 + q
                        mmr(psb[:, 128 * q:128 * (q + 1)],
                            t1m[i][:, 128 * nu:128 * (nu + 1)],
                            W2aug[0:C1, :])
                    nc.vector.tensor_tensor(
                        out=tcat3[:, 4 * half:4 * half + 4,
                                  128 * i:128 * (i + 1)],
                        in0=psb.rearrange("p (q d) -> p q d", d=128),
                        in1=m2fT.rearrange("p (nu d) -> p nu d", d=128)
                        [:, 4 * half:4 * half + 4, :],
                        op=ALU.mult)
            for nu in range(8):
                nc.scalar.copy(tcat3[:, nu, 384:388],
                               p0c4[:, 4 * nu:4 * nu + 4])

            tcatd = nc.dram_tensor("tcatd", [N, CATW], F32).ap()
            wr_inst = nc.sync.dma_start(
                out=tcatd.rearrange("(nu p) w -> p nu w", p=128), in_=tcat3)

            # ---------------- argmax L3 on p0 (DVE-bound; PE may run cold,
            # it still outpaces the DVE chain here)
            m0 = T([128, 8], "m0")
            idxf = T([128, 8], "idxf")
            l3_chunks(m0, idxf, fast=False)

            f0 = T([128, 8], "f0")
            nc.vector.tensor_mul(f0, m0, a3)
            nc.vector.tensor_add(f0, f0, bb3)
            nc.vector.tensor_scalar_max(f0, f0, 0.0)
            dm3 = T([128, 8], "dm3")
            nc.vector.tensor_scalar(dm3, f0, 0.0, None, ALU.is_gt)
            nc.vector.tensor_mul(dm3, dm3, a3)

            idxu32 = T([128, 8], "idxu32", mybir.dt.uint32)
            nc.vector.tensor_copy(idxu32, idxf)

            # ---------------- gathers (Pool desc-gen + DMA) overlapped with
            # the iteration-0 forward (PE/Act/DVE)
            gcs = []
            for c in range(NCH):
                gc = bbp.tile([128, CATW], F32, tag="gc", bufs=8, name="gc")
                gi = nc.gpsimd.indirect_dma_start(
                    out=gc[:, :], out_offset=None, in_=tcatd[:, :],
                    in_offset=bass.IndirectOffsetOnAxis(
                        ap=idxu32[:, c:c + 1], axis=0))
                _tile_mod.add_dep_helper(
                    gi.ins, wr_inst.ins, reason="gather waits table write")
                gcs.append(gc)

            # ---------------- iteration-0 forward with the Jacobian dot
            # products interleaved into the reduce stream (gather c is ready
            # by the time chunk c's reduce lands)
            fjgq = T([128, 24], "fjgq")
            pgs = T([128, 48], "pgs")

            def do_fjgq(c):
                gc = gcs[c]
                # prod[i, d] = gc[:, 128i+d] * W3n[:, 128c+d] then sum over d
                w3c = W3n[:, 128 * c:128 * (c + 1)]
                w3rep = bass.AP(tensor=w3c.tensor, offset=w3c.offset,
                                ap=[list(w3c.ap[0]), [0, 3],
                                    list(w3c.ap[1])])
                prodv = scratch[:, 0:384].rearrange(
                    "p (i d) -> p i d", d=128)
                nc.vector.tensor_tensor(
                    out=prodv,
                    in0=gc[:, 0:384].rearrange("p (i d) -> p i d", d=128),
                    in1=w3rep, op=ALU.mult)
                nc.vector.tensor_reduce(
                    fjgq[:, 3 * c:3 * c + 3], prodv, axis=AX.X, op=ALU.add)
                nc.scalar.copy(pgs[:, 6 * c:6 * c + 3], gc[:, 384:387])
                nc.scalar.copy(pgs[:, 6 * c + 3:6 * c + 6], gc[:, 384:387])

            m = T([128, 8], "m_it")
            fwd12(W1aug, p1hex, fast=False)
            l3_chunks(m, None, fast=False)
            pe_fill(6)
            for c in range(NCH):
                do_fjgq(c)

            # J assembly, vectorized across chunks with strided views
            fjgs = T([128, 48], "fjgs")
            Jt = T([128, 48], "Jt")
            scr48 = T([128, 48], "scr48")
            fjgqv = fjgq.rearrange("p (c i) -> p c i", i=3)
            fjgsv = fjgs.rearrange("p (c i) -> p c i", i=6)
            pgsv = pgs.rearrange("p (c i) -> p c i", i=6)
            Jtv = Jt.rearrange("p (c i) -> p c i", i=6)
            scr48v = scr48.rearrange("p (c i) -> p c i", i=6)
            dm3b = _bcast_free(dm3, 3)  # [p, 8, 0x3]
            nc.vector.tensor_tensor(out=fjgsv[:, :, 0:3], in0=fjgqv,
                                    in1=dm3b, op=ALU.mult)
            nc.vector.tensor_copy(fjgsv[:, :, 3:6], fjgsv[:, :, 0:3])
            # J[:,0:3] = (F2,F0,F1)*(Y,Z,X) - (F1,F2,F0)*(Z,X,Y)
            nc.vector.tensor_tensor(out=Jtv[:, :, 0:3],
                                    in0=fjgsv[:, :, 2:5],
                                    in1=pgsv[:, :, 1:4], op=ALU.mult)
            nc.vector.tensor_tensor(out=scr48v[:, :, 0:3],
                                    in0=fjgsv[:, :, 1:4],
                                    in1=pgsv[:, :, 2:5], op=ALU.mult)
            nc.vector.tensor_tensor(out=Jtv[:, :, 0:3], in0=Jtv[:, :, 0:3],
                                    in1=scr48v[:, :, 0:3], op=ALU.subtract)
            nc.vector.tensor_scalar_mul(Jtv[:, :, 3:6], fjgsv[:, :, 0:3],
                                        -1.0)
            Hp = psm.tile([6, 6], F32, tag="small", name="smallp")
            for c in range(NCH):
                nc.tensor.matmul(Hp, Jt[:, 6 * c:6 * c + 6],
                                 Jt[:, 6 * c:6 * c + 6],
                                 start=(c == 0), stop=(c == NCH - 1))

            # ---------------- 6x6 inverse (unpivoted Gauss-Jordan, SPD)
            M1 = T([6, 12], "M1")
            M2 = T([6, 12], "M2")
            rb6 = T([6, 1], "rb6")
            prow6 = T([6, 12], "prow6")
            ncol = T([6, 1], "ncol")
            nc.scalar.copy(M1[:, 0:6], Hp)
            nc.vector.tensor_copy(M1[:, 6:12], cI6)
            cur, nxt = M1, M2
            for i in range(6):
                pr6 = psm.tile([6, 12], F32, tag="small", name="smallp")
                nc.tensor.matmul(pr6, cE6[:, 6 * i:6 * i + 6], cur)
                nc.vector.reciprocal(rb6, pr6[:, i:i + 1])
                nc.vector.tensor_scalar_mul(prow6, pr6, rb6)
                nc.vector.tensor_scalar_mul(ncol, cur[:, i:i + 1], -1.0)
                nc.vector.tensor_add(ncol, ncol, cI6[:, i:i + 1])
                nc.vector.scalar_tensor_tensor(
                    out=nxt, in0=prow6, scalar=ncol, op0=ALU.mult,
                    in1=cur, op1=ALU.add)
                cur, nxt = nxt, cur
            Hinv = cur[:, 6:12]

            # ---------------- GN iterations
            g_sb = T([4, 4], "g_sb")
            nc.vector.tensor_copy(g_sb, cI4)
            W1g = T([4, C1], "W1g")
            f = T([128, 8], "f_it")
            r = T([128, 8], "r_it")
            tmp48 = T([128, 48], "tmp48")
            s6 = T([128, 6], "s6")
            ysb = T([6, 1], "ysb")
            dxrow = T([1, 6], "dxrow")
            t2sb = T([1, 1], "t2sb")
            scr3 = T([1, 3], "scr3")
            sac3 = T([1, 3], "sac3")
            c3 = T([1, 3], "c3r")
            a13 = T([1, 3], "a13r")
            ww9 = T([1, 9], "ww9")
            W9 = T([1, 9], "W9r")
            R9 = T([1, 9], "R9r")
            Rcols = T([1, 9], "Rcols")
            V9 = T([1, 9], "V9r")
            prod9 = T([1, 9], "prod9")
            RT4 = T([3, 4], "RT4")
            pv4 = T([1, 4], "pv4")
            ET = T([4, 4], "ETm")
            cfill(RT4[:, 3:4], ones=False)
            cfill(pv4[:, 3:4], ones=True)

            def _rv(ap, off, dims):
                # free-dim strided/broadcast view of a [1, x] row AP
                return bass.AP(tensor=ap.tensor, offset=ap.offset + off,
                               ap=[list(ap.ap[0])] + dims)

            ww9v = ww9.rearrange("p (i j) -> p i j", j=3)
            prod9v = prod9.rearrange("p (i j) -> p i j", j=3)
            V9v = V9.rearrange("p (i j) -> p i j", j=3)
            Jtv_ci = Jt.rearrange("p (c i) -> p c i", i=6)
            tmpv_ci = tmp48.rearrange("p (c i) -> p c i", i=6)
            tmpv_ic = tmp48.rearrange("p (c i) -> p i c", i=6)

            for it in range(maxiter):
                if it > 0:
                    wp = small_psum()
                    nc.tensor.matmul(wp[0:4, 0:C1], g_sb, W1aug)
                    nc.scalar.copy(W1g, wp[0:4, 0:C1])
                    fwd12(W1g, p1hex, fast=False)
                    l3_chunks(m, None, fast=False)
                if it < maxiter - 1:
                    pe_fill(1)
                nc.vector.tensor_mul(f, m, a3)
                nc.vector.tensor_add(f, f, bb3)
                if it == maxiter - 1:
                    nc.vector.tensor_scalar_max(f, f, 0.0)
                    nc.vector.tensor_sub(r, f, f0)
                    break
                # r = relu(f) - f0 in one pass
                nc.vector.scalar_tensor_tensor(
                    out=r, in0=f, scalar=0.0, op0=ALU.max,
                    in1=f0, op1=ALU.subtract)
                # y = J^T r via one elementwise pass + strided col-sum + 1 mm
                rb = _bcast_free(r, 6)  # [p, 8, 0x6]
                nc.vector.tensor_tensor(out=tmpv_ci, in0=Jtv_ci, in1=rb,
                                        op=ALU.mult)
                nc.vector.tensor_reduce(s6, tmpv_ic, axis=AX.X, op=ALU.add)
                ycolp = psm.tile([6, 1], F32, tag="small", name="smallp")
                nc.tensor.matmul(ycolp, s6, ones128)
                nc.scalar.copy(ysb, ycolp)
                dxrp = psm.tile([1, 6], F32, tag="small", name="smallp")
                nc.tensor.matmul(dxrp, ysb, Hinv)
                nc.scalar.copy(dxrow, dxrp)
                pe_fill(4)
                # ---- row-form se(3) exp map: one all-DVE block in
                # partition 0, no cross-engine round-trips ----
                wv = dxrow[:, 0:3]
                nc.vector.tensor_tensor(out=scr3, in0=wv, in1=wv,
                                        op=ALU.mult)
                nc.vector.tensor_reduce(t2sb, scr3, axis=AX.X, op=ALU.add)
                # ww9[3i+j] = w_i * w_j
                nc.vector.tensor_tensor(
                    out=ww9v, in0=_rv(dxrow, 0, [[1, 3], [0, 3]]),
                    in1=_rv(dxrow, 0, [[0, 3], [1, 3]]), op=ALU.mult)
                # W9 = x*S0 + y*S1 + z*S2 (row-major skew)
                nc.vector.tensor_scalar_mul(W9, cS9r[:, 0:9],
                                            dxrow[:, 0:1])
                nc.vector.scalar_tensor_tensor(
                    out=W9, in0=cS9r[:, 9:18], scalar=dxrow[:, 1:2],
                    op0=ALU.mult, in1=W9, op1=ALU.add)
                nc.vector.scalar_tensor_tensor(
                    out=W9, in0=cS9r[:, 18:27], scalar=dxrow[:, 2:3],
                    op0=ALU.mult, in1=W9, op1=ALU.add)
                # series s1,s2,s3: Horner in t2, degree 4 (|t2|<~0.05)
                nc.vector.tensor_copy(sac3, cser[:, 6:9])
                for j in range(3, 7):
                    nc.vector.scalar_tensor_tensor(
                        out=sac3, in0=sac3, scalar=t2sb,
                        op0=ALU.mult, in1=cser[:, 3 * j:3 * j + 3],
                        op1=ALU.add)
                # a13 = 1 - t2*(s1,s2,s3)
                nc.vector.tensor_scalar_mul(c3, sac3, t2sb)
                nc.vector.scalar_tensor_tensor(
                    out=a13, in0=c3, scalar=-1.0, op0=ALU.mult,
                    in1=cones[0:1, 0:3], op1=ALU.add)
                # R = s1*W + (1-s2 t2)*I + s2*wwT ; V likewise with s2,s3
                nc.vector.tensor_scalar_mul(R9, W9, sac3[:, 0:1])
                nc.vector.scalar_tensor_tensor(
                    out=R9, in0=cI9r, scalar=a13[:, 1:2], op0=ALU.mult,
                    in1=R9, op1=ALU.add)
                nc.vector.scalar_tensor_tensor(
                    out=R9, in0=ww9, scalar=sac3[:, 1:2], op0=ALU.mult,
                    in1=R9, op1=ALU.add)
                nc.vector.tensor_scalar_mul(V9, W9, sac3[:, 1:2])
                nc.vector.scalar_tensor_tensor(
                    out=V9, in0=cI9r, scalar=a13[:, 2:3], op0=ALU.mult,
                    in1=V9, op1=ALU.add)
                nc.vector.scalar_tensor_tensor(
                    out=V9, in0=ww9, scalar=sac3[:, 2:3], op0=ALU.mult,
                    in1=V9, op1=ALU.add)
                # pv4[0:3] = V @ v
                nc.vector.tensor_tensor(
                    out=prod9v, in0=V9v,
                    in1=_rv(dxrow, 3, [[0, 3], [1, 3]]), op=ALU.mult)
                nc.vector.tensor_reduce(pv4[:, 0:3], prod9v, axis=AX.X,
                                        op=ALU.add)
                # Rcols = R9 transposed (col-major) via one strided DVE copy
                nc.vector.tensor_copy(
                    _rv(Rcols, 0, [[1, 3], [3, 3]]),
                    _rv(R9, 0, [[3, 3], [1, 3]]))
                # RT rows (= R columns) via 3 PE outer products e_k (x) R[:,k]
                rtp = psm.tile([3, 3], F32, tag="small", name="smallp")
                for k2 in range(3):
                    nc.tensor.matmul(rtp, cI9r[:, 3 * k2:3 * k2 + 3],
                                     Rcols[:, 3 * k2:3 * k2 + 3],
                                     start=(k2 == 0), stop=(k2 == 2))
                nc.scalar.copy(RT4[:, 0:3], rtp)
                etp = psm.tile([4, 4], F32, tag="small", name="smallp")
                nc.tensor.matmul(etp, cI34, RT4, start=True, stop=False)
                nc.tensor.matmul(etp, cE4, pv4, start=False, stop=True)
                nc.scalar.copy(ET, etp)
                # G_new = E @ G = (ET)^T @ g_sb -- no transpose needed
                g2p = psm.tile([4, 4], F32, tag="small", name="smallp")
                nc.tensor.matmul(g2p, ET, g_sb)
                nc.scalar.copy(g_sb, g2p)

            nc.sync.dma_start(
                out=outd.rearrange("(c p) -> p c", p=128), in_=r)

    _split_multi_waits(nc)
    return nc


# ---------------------------------------------------------------- host side
def _pack_params(inputs):
    # cols: [0:10] gamma(l1,l2,l3k*8), [10:20] beta, [20:30] rm, [30:40] rv,
    # [40:50] conv-bias; layer3 vectors in k-chunk layout
    prm = np.zeros((128, 50), np.float32)
    for g, grp in enumerate([("gamma1", "gamma2", "gamma3"),
                             ("beta1", "beta2", "beta3"),
                             ("rm1", "rm2", "rm3"),
                             ("rv1", "rv2", "rv3"),
                             ("b1", "b2", "b3")]):
        base = 10 * g
        prm[:C1, base + 0] = np.asarray(inputs[grp[0]], np.float32)
        prm[:C2, base + 1] = np.asarray(inputs[grp[1]], np.float32)
        v = np.asarray(inputs[grp[2]], np.float32).reshape(8, 128)
        prm[:, base + 2:base + 10] = v.T
    return prm


def make_in_maps(inputs):
    B = int(np.asarray(inputs["p0"]).shape[0])
    prm = _pack_params(inputs)
    W2T = np.ascontiguousarray(np.asarray(inputs["W2"], np.float32).T)
    W3n = np.ascontiguousarray(np.asarray(inputs["W3"], np.float32))
    W3T = np.ascontiguousarray(W3n.T)
    p0 = np.asarray(inputs["p0"], np.float32)
    p1 = np.asarray(inputs["p1"], np.float32)
    W1 = np.ascontiguousarray(np.asarray(inputs["W1"], np.float32))
    return [
        {
            "p0": np.ascontiguousarray(p0[b]),
            "p1": np.ascontiguousarray(p1[b]),
            "W1": W1,
            "W2T": W2T,
            "W3T": W3T,
            "W3": W3n,
            "prm": prm,
            "consts": CONSTS,
        }
        for b in range(B)
    ]


_NC_CACHE = {}
TRACE = False
LAST_RESULT = None
USE_F32R = False
REDUCE_TTR = True


def kernel(**inputs):
    global LAST_RESULT
    maxiter = int(np.asarray(inputs["maxiter"]))
    B = int(np.asarray(inputs["p0"]).shape[0])
    if maxiter <= 0:
        return np.zeros((B, K), np.float32)

    key = (maxiter, USE_F32R, REDUCE_TTR)
    if key not in _NC_CACHE:
        _NC_CACHE[key] = build_kernel(maxiter, use_f32r=USE_F32R,
                                      reduce_ttr=REDUCE_TTR)
    nc = _NC_CACHE[key]
    in_maps = make_in_maps(inputs)
    res = run_bass_kernel_spmd(nc, in_maps, core_ids=list(range(B)),
                               trace=TRACE)
    LAST_RESULT = res
    return np.stack([res.results[b]["out"] for b in range(B)], axis=0)



# revision 47
# speedup vs baseline: 1.1430x; 1.1430x over previous
"""AnalyticalPointNetLK forward on 8 Trainium2 NeuronCores.

Pure data parallel: batch element b -> core b. Everything (PointNet forward,
argmax, analytical Jacobian, 6x6 inverse, Gauss-Newton iterations, se(3) exp
map) runs on-device in one NEFF.

Self-contained: only needs the container's concourse stack.
"""
import sys
import types

import numpy as np


# ---------------------------------------------------------------- env setup
def _setup_env():
    try:
        import concourse.bass  # noqa: F401
    except ImportError:
        sys.path.insert(0, "/opt/trn_rl_repo")

    # Inject antenv.axon_hooks (missing in this image) so trace=True works.
    try:
        import antenv

        if not hasattr(antenv, "axon_hooks"):
            _m = types.ModuleType("antenv.axon_hooks")
            _m._hook = None
            _m.set_axon_ntff_profile_hook = lambda h: setattr(_m, "_hook", h)
            _m.get_axon_ntff_profile_hook = lambda: _m._hook
            sys.modules["antenv.axon_hooks"] = _m
            antenv.axon_hooks = _m
            try:
                from trn_agent_boot.trn_boot import _ntff_profile_via_ctypes

                h = _ntff_profile_via_ctypes("/opt/axon/libaxon_pjrt.so")
                if h is not None:
                    _m.set_axon_ntff_profile_hook(h)
            except Exception:
                pass
    except ImportError:
        pass

    # Split multi-wait exit Drain (this walrus rejects >1 sem wait on CTRL).
    from concourse.tile import TileContext
    from concourse.vector_clock import ScopedClock

    if not getattr(TileContext, "_drain_split_installed", False):

        def _patched(self, tick_clock, wait_clock):
            nc = self.nc
            drain_inst = nc.sync.drain()
            wait_clock.add_sem_waits(
                drain_inst.ins, ScopedClock({None: tick_clock.global_clock})
            )
            si = drain_inst.ins.sync_info
            if si is not None and si.on_wait and len(si.on_wait) > 1:
                waits = list(si.on_wait)
                si.on_wait = waits[:1]
                for w in waits[1:]:
                    extra = nc.sync.drain()
                    esi = extra.ins.sync_info
                    if esi is None:
                        import bass_rust

                        extra.ins.sync_info = bass_rust.SyncInfo(
                            on_wait=[w], on_update=[]
                        )
                    else:
                        esi.on_wait = [w]
            nc.all_engine_barrier()
            assert self.sems is not None
            popped = nc._tile_sem_poison_stack.pop()
            assert popped is self._sem_poison
            nc.clear_and_free_semaphores(list(self.sems.allocated().values()))
            nc.all_engine_barrier()

        TileContext._drain_and_barrier = _patched
        TileContext._drain_split_installed = True


_setup_env()

import concourse.bass as bass  # noqa: E402
import concourse.mybir as mybir  # noqa: E402
from concourse.tile import TileContext  # noqa: E402
from concourse.bass_utils import run_bass_kernel_spmd  # noqa: E402

F32 = mybir.dt.float32
F32R = mybir.dt.float32r
ALU = mybir.AluOpType
ACTF = mybir.ActivationFunctionType
AX = mybir.AxisListType

N, K, C1, C2 = 1024, 1024, 64, 128
NCH = 8
EPS = 1e-5
NEG_INF = -3.0e38

# exp-map series (Horner, highest degree first): s1=sin t/t, s2=(1-cos t)/t^2,
# s3=(t-sin t)/t^3 as series in t2=t^2, up to t^12
SER = np.array(
    [
        [1 / 6227020800, 1 / 87178291200, 1 / 1307674368000],
        [-1 / 39916800, -1 / 479001600, -1 / 6227020800],
        [1 / 362880, 1 / 3628800, 1 / 39916800],
        [-1 / 5040, -1 / 40320, -1 / 362880],
        [1 / 120, 1 / 720, 1 / 5040],
        [-1 / 6, -1 / 24, -1 / 120],
        [1.0, 0.5, 1 / 6],
    ],
    np.float32,
)

# ------------------------------------------------------------- consts blob
OFF_IDENT = 0
OFF_IOTA = OFF_IDENT + 128 * 128
OFF_SER = OFF_IOTA + 1024
OFF_S = OFF_SER + 21          # S0|S1|S2 skew generators, [3 rows x 9]
OFF_I3 = OFF_S + 27           # [3 x 3]
OFF_I34 = OFF_I3 + 9          # [3 x 4]
OFF_E4 = OFF_I34 + 12         # [1 x 4]
OFF_I4 = OFF_E4 + 4           # [4 x 4]
OFF_E16 = OFF_I4 + 16         # [16 x 3]
OFF_ONES = OFF_E16 + 48       # [6 x 8] ones
OFF_E6 = OFF_ONES + 48        # [6 x 36]: cE6[:, 6i:6i+6] = row-i-ones matrix
OFF_I6 = OFF_E6 + 216         # [6 x 6]
OFF_I9ROW = OFF_I6 + 36       # [1 x 9] I3 row-major
OFF_S9ROW = OFF_I9ROW + 9     # [1 x 27] skew generators row-major
OFF_ONE1K = OFF_S9ROW + 27    # [1024] ones
OFF_ZERO1K = OFF_ONE1K + 1024  # [1024] zeros
CONST_LEN = OFF_ZERO1K + 1024


def _build_consts():
    c = np.zeros(CONST_LEN, np.float32)
    c[OFF_IDENT:OFF_IDENT + 128 * 128] = np.eye(128, dtype=np.float32).ravel()
    c[OFF_IOTA:OFF_IOTA + 1024] = np.arange(1024, dtype=np.float32)
    c[OFF_SER:OFF_SER + 21] = SER.ravel()
    S0 = np.array([[0, 0, 0], [0, 0, -1], [0, 1, 0]], np.float32)
    S1 = np.array([[0, 0, 1], [0, 0, 0], [-1, 0, 0]], np.float32)
    S2 = np.array([[0, -1, 0], [1, 0, 0], [0, 0, 0]], np.float32)
    c[OFF_S:OFF_S + 27] = np.concatenate([S0, S1, S2], axis=1).ravel()
    c[OFF_I3:OFF_I3 + 9] = np.eye(3, dtype=np.float32).ravel()
    c[OFF_I34:OFF_I34 + 12] = np.eye(3, 4, dtype=np.float32).ravel()
    c[OFF_E4:OFF_E4 + 4] = np.array([0, 0, 0, 1], np.float32)
    c[OFF_I4:OFF_I4 + 16] = np.eye(4, dtype=np.float32).ravel()
    c[OFF_E16:OFF_E16 + 48] = np.eye(16, 3, dtype=np.float32).ravel()
    c[OFF_ONES:OFF_ONES + 48] = 1.0
    e6 = np.zeros((6, 36), np.float32)
    for i in range(6):
        e6[i, 6 * i:6 * i + 6] = 1.0
    c[OFF_E6:OFF_E6 + 216] = e6.ravel()
    c[OFF_I6:OFF_I6 + 36] = np.eye(6, dtype=np.float32).ravel()
    c[OFF_I9ROW:OFF_I9ROW + 9] = np.eye(3, dtype=np.float32).ravel()
    c[OFF_S9ROW:OFF_S9ROW + 27] = np.concatenate(
        [np.array([[0, 0, 0], [0, 0, -1], [0, 1, 0]], np.float32).ravel(),
         np.array([[0, 0, 1], [0, 0, 0], [-1, 0, 0]], np.float32).ravel(),
         np.array([[0, -1, 0], [1, 0, 0], [0, 0, 0]], np.float32).ravel()])
    c[OFF_ONE1K:OFF_ONE1K + 1024] = 1.0
    return c


CONSTS = _build_consts()


def _split_multi_waits(nc):
    """walrus (this build) accepts at most one sem wait per instruction on
    several opcode classes. Split any instruction with >1 waits by inserting
    same-engine nops, each carrying one wait, immediately before it."""
    import bass_rust

    def _make_nop(engine):
        h = nc.engines[engine]
        inst = h.nop(nofuse=True)
        # nop() appended to the current bb; detach it
        for f in nc.m.functions:
            for bb in f.blocks:
                lst = bb.instructions
                if lst and lst[-1] is inst.ins:
                    lst.pop()
                    return inst.ins
        raise RuntimeError("nop not found for detach")

    for f in nc.m.functions:
        for bb in f.blocks:
            lst = bb.instructions
            out = []
            changed = False
            for inst in list(lst):
                si = inst.sync_info
                if si is not None and si.on_wait and len(si.on_wait) > 1:
                    waits = list(si.on_wait)
                    for w in waits[:-1]:
                        nop = _make_nop(inst.engine)
                        nop.sync_info = bass_rust.SyncInfo(
                            on_wait=[w], on_update=[])
                        out.append(nop)
                    si.on_wait = [waits[-1]]
                    changed = True
                out.append(inst)
            if changed:
                lst.clear()
                lst.extend(out)


def _bcast_free(ap, count):
    """Append a stride-0 free dim (broadcast) to an AP view."""
    dims = [list(d) for d in ap.ap] + [[0, count]]
    return bass.AP(tensor=ap.tensor, offset=ap.offset, ap=dims)


def build_kernel(maxiter: int, use_f32r: bool = False, reduce_ttr: bool = True,
                 debug: bool = False):
    nc = bass.Bass("TRN2", target_bir_lowering=False, debug=False,
                   num_devices=8)

    def din(name, shape, dtype=F32):
        return nc.dram_tensor(name, shape, dtype, kind="ExternalInput").ap()

    p0d = din("p0", [N, 3])
    p1d = din("p1", [N, 3])
    W1d = din("W1", [C1, 3])
    W2Td = din("W2T", [C1, C2])
    W3Td = din("W3T", [C2, K])
    W3nd = din("W3", [K, C2])
    prmd = din("prm", [128, 50])
    constsd = din("consts", [CONST_LEN])
    outd = nc.dram_tensor("out", [K], F32, kind="ExternalOutput").ap()

    with TileContext(nc) as tc:
        with (
            tc.tile_pool(name="per", bufs=1) as per,
            tc.tile_pool(name="pbig", bufs=2, space="PSUM") as pbig,
            tc.tile_pool(name="psm", bufs=2, space="PSUM") as psm,
            tc.tile_pool(name="pfill", bufs=1, space="PSUM") as pfill,
            tc.tile_pool(name="bbp", bufs=2) as bbp,
            tc.tile_pool(name="dramp", bufs=1, space="DRAM") as dramp,
        ):
            def T(shape, tag, dtype=F32):
                return per.tile(shape, dtype, tag=tag, name=tag)

            def big_psum():
                return pbig.tile([128, 1024], F32, tag="big", name="bigp")

            def small_psum():
                return psm.tile([128, 64], F32, tag="small", name="smallp")

            def mmr(out, lhsT, rhs, fast=None, **kw):
                f = use_f32r if fast is None else (fast and use_f32r)
                if f:
                    nc.tensor.matmul(out, lhsT.bitcast(F32R),
                                     rhs.bitcast(F32R), **kw)
                else:
                    nc.tensor.matmul(out, lhsT, rhs, **kw)

            def rr(ap):
                # producers feeding an fp32r matmul must round to fp32r
                return ap.bitcast(F32R) if use_f32r else ap

            def cfill(dst, ones: bool, f32r: bool = False):
                # memset replacement (this walrus rejects InstMemset):
                # broadcast-DMA a constant vector from the consts blob
                p, fsz = dst.shape[0], 1
                for d in dst.shape[1:]:
                    fsz *= d
                assert fsz <= 1024
                off = OFF_ONE1K if ones else OFF_ZERO1K
                srcap = bass.AP(tensor=constsd.tensor,
                                offset=constsd.offset + off,
                                ap=[[0, p], [1, fsz]])
                if f32r and use_f32r:
                    srcap = srcap.bitcast(F32R)
                    dst = rr(dst)
                dmax(
                    out=dst.rearrange(
                        " ".join(f"d{i}" for i in range(len(dst.shape)))
                        + " -> d0 ("
                        + " ".join(f"d{i}" for i in range(1, len(dst.shape)))
                        + ")") if len(dst.shape) > 2 else dst,
                    in_=srcap)

            # ---------------- const + input DMAs
            # spread descriptor generation across four otherwise-idle
            # sequencers -- a single queue serializes ~25 input DMAs into
            # ~22us of setup
            _dmaq = [nc.sync, nc.gpsimd, nc.scalar]
            _dmai = [0]

            def dmax(out, in_):
                e = _dmaq[_dmai[0] % len(_dmaq)]
                _dmai[0] += 1
                e.dma_start(out=out, in_=in_)

            def cdma(shape, tag, off, length):
                t = T(shape, tag)
                dmax(out=t,
                     in_=constsd[off:off + length].rearrange(
                         "(p f) -> p f", p=shape[0]))
                return t

            ident = cdma([128, 128], "ident", OFF_IDENT, 128 * 128)
            cser = cdma([1, 21], "cser", OFF_SER, 21)
            cE4 = cdma([1, 4], "cE4", OFF_E4, 4)
            cI4 = cdma([4, 4], "cI4", OFF_I4, 16)
            cones = cdma([6, 8], "cones", OFF_ONES, 48)
            cE6 = cdma([6, 36], "cE6", OFF_E6, 216)
            cI6 = cdma([6, 6], "cI6", OFF_I6, 36)
            cI9r = cdma([1, 9], "cI9r", OFF_I9ROW, 9)
            cS9r = cdma([1, 27], "cS9r", OFF_S9ROW, 27)

            prm = T([128, 50], "prm")
            dmax(out=prm, in_=prmd[:, :])
            W1sb = T([C1, 3], "W1sb")
            dmax(out=W1sb, in_=W1d[:, :])
            W2T = T([C1, C2], "W2T")
            dmax(out=rr(W2T), in_=W2Td[:, :].bitcast(F32R)
                 if use_f32r else W2Td[:, :])
            p0c4 = T([128, 32], "p0c")
            cfill(p0c4, ones=True)
            dmax(out=p0c4.rearrange("p (k d) -> p k d", d=4)[:, :, 0:3],
                 in_=p0d.rearrange("(p k) d -> p k d", p=128))
            p1c = T([128, 32], "p1c")
            cfill(p1c, ones=True)
            dmax(out=p1c.rearrange("p (k d) -> p k d", d=4)[:, :, 0:3],
                 in_=p1d.rearrange("(p k) d -> p k d", p=128))
            W3T = T([C2, K], "W3T")
            dmax(out=rr(W3T), in_=W3Td[:, :].bitcast(F32R)
                 if use_f32r else W3Td[:, :])
            W3n = T([128, 1024], "W3n")
            w3n_src = bass.AP(tensor=W3nd.tensor, offset=W3nd.offset,
                              ap=[[128, 128], [16384, 8], [1, 128]])
            dmax(out=W3n.rearrange("p (c d) -> p c d", d=128), in_=w3n_src)
            ones128 = T([128, 1], "ones128")
            cfill(ones128, ones=True)

            # ---------------- param prep: a = gamma*rsqrt(rv+eps),
            # bb = a*(b-rm)+beta  (rsqrt: exact sqrt + bit-exact reciprocal)
            def bn_fold(gam, bet, rm, rv, bias, pshape, tagp):
                t = T(pshape, tagp + "_t")
                nc.vector.tensor_scalar_add(t, rv, EPS)
                s = T(pshape, tagp + "_s")
                nc.scalar.sqrt(s, t)
                y = T(pshape, tagp + "_y")
                nc.vector.reciprocal(y, s)
                a = T(pshape, tagp + "_a")
                nc.vector.tensor_mul(a, gam, y)
                bb = T(pshape, tagp + "_bb")
                nc.vector.tensor_sub(bb, bias, rm)
                nc.vector.tensor_mul(bb, bb, a)
                nc.vector.tensor_add(bb, bb, bet)
                return a, bb

            ablk, bbblk = bn_fold(prm[:, 0:10], prm[:, 10:20],
                                  prm[:, 20:30], prm[:, 30:40],
                                  prm[:, 40:50], [128, 10], "bn")
            a1, bb1 = ablk[0:C1, 0:1], bbblk[0:C1, 0:1]
            a2, bb2 = ablk[0:C2, 1:2], bbblk[0:C2, 1:2]
            a3, bb3 = ablk[:, 2:10], bbblk[:, 2:10]

            # ---------------- weight prep
            W1s = T([C1, 3], "W1s")
            nc.vector.tensor_scalar_mul(W1s, W1sb, a1)
            W1s4 = T([C1, 4], "W1s4")
            nc.vector.tensor_copy(W1s4[:, 0:3], W1s)
            nc.vector.tensor_copy(W1s4[:, 3:4], bb1)
            W1aug = T([4, C1], "W1aug")
            tp = small_psum()
            nc.tensor.transpose(tp[0:4, 0:C1], W1s4, ident[0:C1, 0:C1])
            nc.scalar.copy(rr(W1aug), tp[0:4, 0:C1])

            W2aug = T([C1 + 1, C2], "W2aug")
            a2dram = dramp.tile([C2], F32, tag="a2d", name="a2d")
            nc.sync.dma_start(out=a2dram, in_=a2)
            a2bc = T([C1, C2], "a2bc")
            a2bc_src = bass.AP(tensor=a2dram.tensor, offset=a2dram.offset,
                               ap=[[0, C1], [1, C2]])
            nc.sync.dma_start(out=a2bc, in_=a2bc_src)
            nc.vector.tensor_mul(rr(W2aug[0:C1, :]), W2T, a2bc)
            tp4 = psm.tile([128, 128], F32, tag="small", name="smallp")
            nc.tensor.transpose(tp4[0:1, 0:C2], bb2, ident[0:C2, 0:C2])
            nc.scalar.copy(rr(W2aug[C1:C1 + 1, :]), tp4[0:1, 0:C2])

            # ---------------- p0 / p1 transposed homogeneous
            p0hex = T([16, N], "p0hex")
            p1hex = T([4, N], "p1hex")
            cfill(p0hex, ones=False, f32r=True)
            for (srct, dsttile) in ((p0c4, p0hex), (p1c, p1hex)):
                tpp = big_psum()
                for c in range(8):
                    nc.tensor.transpose(
                        tpp[0:4, c * 128:(c + 1) * 128],
                        srct[:, c * 4:(c + 1) * 4], ident)
                nc.vector.tensor_copy(rr(dsttile[0:4, :]), tpp[0:4, 0:N])

            iotabc = T([128, 1024], "iotabc")
            iot_src = bass.AP(tensor=constsd.tensor,
                              offset=constsd.offset + OFF_IOTA,
                              ap=[[0, 128], [1, 1024]])
            dmax(out=iotabc, in_=iot_src)

            # ---------------- shared tiles
            scratch = T([128, 1024], "scratch")
            x1s = T([C1 + 1, N], "x1s")
            cfill(x1s[C1:C1 + 1, :], ones=True)
            x2s = T([C2, N], "x2s")

            # PE-warmth fillers: junk fp32 matmuls on constant tiles, issued
            # into per-iteration PE idle windows so the tensor clock stays
            # at its top p-state (cold restarts run ~1.8x slower)
            def pe_fill(n):
                for _ in range(n):
                    fj = pfill.tile([128, 512], F32, tag="fill",
                                    name="fillp")
                    nc.tensor.matmul(fj, W3T[:, 0:128], W3n[:, 0:512])

            def fwd12(lhsT1, phex, fast=True):
                u1 = big_psum()
                nc.tensor.matmul(u1[0:C1, 0:512], lhsT1, phex[0:4, 0:512])
                nc.tensor.matmul(u1[0:C1, 512:1024], lhsT1,
                                 phex[0:4, 512:1024])
                nc.scalar.activation(x1s[0:C1, 0:512], u1[0:C1, 0:512],
                                     ACTF.Relu)
                nc.scalar.activation(x1s[0:C1, 512:1024],
                                     u1[0:C1, 512:1024], ACTF.Relu)
                u2 = big_psum()
                nc.tensor.matmul(u2[0:C2, 0:512], W2aug, x1s[:, 0:512])
                nc.tensor.matmul(u2[0:C2, 512:1024], W2aug,
                                 x1s[:, 512:1024])
                nc.scalar.activation(x2s[:, 0:512], u2[0:C2, 0:512],
                                     ACTF.Relu)
                nc.scalar.activation(x2s[:, 512:1024],
                                     u2[0:C2, 512:1024], ACTF.Relu)

            # chunks routed PSUM->SBUF via the idle Act engine: frees the
            # PSUM tile early and lets the DVE reduce run from SBUF (no
            # 120-cycle PSUM bubble, 2x port mode when available)
            xsb = [T([128, 1024], f"xsb{i}") for i in range(2)]
            ACT_CHUNKS = (2, 4, 6)

            def l3_chunks(m_out, idx_out, fast=True, after_chunk=None):
                for c in range(NCH):
                    ps = big_psum()
                    lh = W3T[:, c * 128:(c + 1) * 128]
                    nc.tensor.matmul(ps[0:C2, 0:512], lh, x2s[:, 0:512])
                    nc.tensor.matmul(ps[0:C2, 512:1024], lh,
                                     x2s[:, 512:1024])
                    if c in ACT_CHUNKS:
                        src = xsb[ACT_CHUNKS.index(c) % 2]
                        nc.scalar.copy(src[:, 0:512], ps[:, 0:512])
                        nc.scalar.copy(src[:, 512:1024], ps[:, 512:1024])
                    else:
                        src = ps
                    nc.vector.tensor_reduce(
                        m_out[:, c:c + 1], src[:, 0:1024], axis=AX.X,
                        op=ALU.max)
                    if idx_out is not None:
                        nc.vector.scalar_tensor_tensor(
                            out=scratch, in0=src[:, 0:1024],
                            scalar=m_out[:, c:c + 1], in1=iotabc,
                            op0=ALU.is_ge, op1=ALU.mult,
                            accum_out=idx_out[:, c:c + 1])
                    if after_chunk is not None:
                        after_chunk(c)

            # ---------------- setup forward on p0 (masks + argmax)
            fwd12(W1aug, p0hex, fast=False)

            # masks via Act sign (x>=0 post-relu, so sign == is_gt-0 mask);
            # keeps the DVE free for the argmax phase
            m1f = T([C1, N], "m1f")
            nc.scalar.sign(m1f, x1s[0:C1, :])
            m2f = T([C2, N], "m2f")
            nc.scalar.sign(m2f, x2s)

            # ---------------- gather-table build (idx-independent, emitted
            # before the argmax L3 so its PE/DMA work overlaps DVE reduces)
            import concourse.tile as _tile_mod

            # m2f transposed: m2fT[:, 128*nu:...] = m2f chunk nu ^T
            m2fT = T([128, 1024], "m2fT")
            for nu in range(8):
                tpm = psm.tile([128, 128], F32, tag="small", name="smallp")
                nc.tensor.transpose(tpm, m2f[:, 128 * nu:128 * (nu + 1)],
                                    ident)
                nc.scalar.copy(m2fT[:, 128 * nu:128 * (nu + 1)], tpm)

            t1m = []
            for i in range(3):
                t = T([C1, N], f"t1m{i}")
                nc.scalar.mul(rr(t), m1f, W1s[:, i:i + 1])
                t1m.append(t)

            CATW = 388
            tcat = T([128, 8 * CATW], "tcat")
            tcat3 = tcat.rearrange("p (nu w) -> p nu w", w=CATW)
            # t2mT chunks: psum[128n, 512] holds nu-batch of 4 for one i
            for i in range(3):
                for half in range(2):
                    psb = pbig.tile([128, 512], F32, tag="big", name="ttp")
                    for q in range(4):
                        nu = half * 4 + q
                        mmr(psb[:, 128 * q:128 * (q + 1)],
                            t1m[i][:, 128 * nu:128 * (nu + 1)],
                            W2aug[0:C1, :])
                    nc.vector.tensor_tensor(
                        out=tcat3[:, 4 * half:4 * half + 4,
                                  128 * i:128 * (i + 1)],
                        in0=psb.rearrange("p (q d) -> p q d", d=128),
                        in1=m2fT.rearrange("p (nu d) -> p nu d", d=128)
                        [:, 4 * half:4 * half + 4, :],
                        op=ALU.mult)
            for nu in range(8):
                nc.scalar.copy(tcat3[:, nu, 384:388],
                               p0c4[:, 4 * nu:4 * nu + 4])

            tcatd = nc.dram_tensor("tcatd", [N, CATW], F32).ap()
            wr_inst = nc.sync.dma_start(
                out=tcatd.rearrange("(nu p) w -> p nu w", p=128), in_=tcat3)

            # ---------------- argmax L3 on p0 (DVE-bound; PE may run cold,
            # it still outpaces the DVE chain here)
            m0 = T([128, 8], "m0")
            idxf = T([128, 8], "idxf")
            l3_chunks(m0, idxf, fast=False)

            f0 = T([128, 8], "f0")
            nc.vector.tensor_mul(f0, m0, a3)
            nc.vector.tensor_add(f0, f0, bb3)
            nc.vector.tensor_scalar_max(f0, f0, 0.0)
            dm3 = T([128, 8], "dm3")
            nc.vector.tensor_scalar(dm3, f0, 0.0, None, ALU.is_gt)
            nc.vector.tensor_mul(dm3, dm3, a3)

            idxu32 = T([128, 8], "idxu32", mybir.dt.uint32)
            nc.vector.tensor_copy(idxu32, idxf)

            # ---------------- gathers (Pool desc-gen + DMA) overlapped with
            # the iteration-0 forward (PE/Act/DVE)
            gcs = []
            for c in range(NCH):
                gc = bbp.tile([128, CATW], F32, tag="gc", bufs=8, name="gc")
                gi = nc.gpsimd.indirect_dma_start(
                    out=gc[:, :], out_offset=None, in_=tcatd[:, :],
                    in_offset=bass.IndirectOffsetOnAxis(
                        ap=idxu32[:, c:c + 1], axis=0))
                _tile_mod.add_dep_helper(
                    gi.ins, wr_inst.ins, reason="gather waits table write")
                gcs.append(gc)

            # ---------------- iteration-0 forward with the Jacobian dot
            # products interleaved into the reduce stream (gather c is ready
            # by the time chunk c's reduce lands)
            fjgq = T([128, 24], "fjgq")
            pgs = T([128, 48], "pgs")

            def do_fjgq(c):
                gc = gcs[c]
                # prod[i, d] = gc[:, 128i+d] * W3n[:, 128c+d] then sum over d
                w3c = W3n[:, 128 * c:128 * (c + 1)]
                w3rep = bass.AP(tensor=w3c.tensor, offset=w3c.offset,
                                ap=[list(w3c.ap[0]), [0, 3],
                                    list(w3c.ap[1])])
                prodv = scratch[:, 0:384].rearrange(
                    "p (i d) -> p i d", d=128)
                nc.vector.tensor_tensor(
                    out=prodv,
                    in0=gc[:, 0:384].rearrange("p (i d) -> p i d", d=128),
                    in1=w3rep, op=ALU.mult)
                nc.vector.tensor_reduce(
                    fjgq[:, 3 * c:3 * c + 3], prodv, axis=AX.X, op=ALU.add)
                nc.scalar.copy(pgs[:, 6 * c:6 * c + 3], gc[:, 384:387])
                nc.scalar.copy(pgs[:, 6 * c + 3:6 * c + 6], gc[:, 384:387])

            m = T([128, 8], "m_it")
            fwd12(W1aug, p1hex, fast=False)
            l3_chunks(m, None, fast=False)
            pe_fill(6)
            for c in range(NCH):
                do_fjgq(c)

            # J assembly, vectorized across chunks with strided views
            fjgs = T([128, 48], "fjgs")
            Jt = T([128, 48], "Jt")
            scr48 = T([128, 48], "scr48")
            fjgqv = fjgq.rearrange("p (c i) -> p c i", i=3)
            fjgsv = fjgs.rearrange("p (c i) -> p c i", i=6)
            pgsv = pgs.rearrange("p (c i) -> p c i", i=6)
            Jtv = Jt.rearrange("p (c i) -> p c i", i=6)
            scr48v = scr48.rearrange("p (c i) -> p c i", i=6)
            dm3b = _bcast_free(dm3, 3)  # [p, 8, 0x3]
            nc.vector.tensor_tensor(out=fjgsv[:, :, 0:3], in0=fjgqv,
                                    in1=dm3b, op=ALU.mult)
            nc.vector.tensor_copy(fjgsv[:, :, 3:6], fjgsv[:, :, 0:3])
            # J[:,0:3] = (F2,F0,F1)*(Y,Z,X) - (F1,F2,F0)*(Z,X,Y)
            nc.vector.tensor_tensor(out=Jtv[:, :, 0:3],
                                    in0=fjgsv[:, :, 2:5],
                                    in1=pgsv[:, :, 1:4], op=ALU.mult)
            nc.vector.tensor_tensor(out=scr48v[:, :, 0:3],
                                    in0=fjgsv[:, :, 1:4],
                                    in1=pgsv[:, :, 2:5], op=ALU.mult)
            nc.vector.tensor_tensor(out=Jtv[:, :, 0:3], in0=Jtv[:, :, 0:3],
                                    in1=scr48v[:, :, 0:3], op=ALU.subtract)
            nc.vector.tensor_scalar_mul(Jtv[:, :, 3:6], fjgsv[:, :, 0:3],
                                        -1.0)
            Hp = psm.tile([6, 6], F32, tag="small", name="smallp")
            for c in range(NCH):
                nc.tensor.matmul(Hp, Jt[:, 6 * c:6 * c + 6],
                                 Jt[:, 6 * c:6 * c + 6],
                                 start=(c == 0), stop=(c == NCH - 1))

            # ---------------- 6x6 inverse (unpivoted Gauss-Jordan, SPD)
            M1 = T([6, 12], "M1")
            M2 = T([6, 12], "M2")
            rb6 = T([6, 1], "rb6")
            prow6 = T([6, 12], "prow6")
            ncol = T([6, 1], "ncol")
            nc.scalar.copy(M1[:, 0:6], Hp)
            nc.vector.tensor_copy(M1[:, 6:12], cI6)
            cur, nxt = M1, M2
            for i in range(6):
                pr6 = psm.tile([6, 12], F32, tag="small", name="smallp")
                nc.tensor.matmul(pr6, cE6[:, 6 * i:6 * i + 6], cur)
                nc.vector.reciprocal(rb6, pr6[:, i:i + 1])
                nc.vector.tensor_scalar_mul(prow6, pr6, rb6)
                nc.vector.tensor_scalar_mul(ncol, cur[:, i:i + 1], -1.0)
                nc.vector.tensor_add(ncol, ncol, cI6[:, i:i + 1])
                nc.vector.scalar_tensor_tensor(
                    out=nxt, in0=prow6, scalar=ncol, op0=ALU.mult,
                    in1=cur, op1=ALU.add)
                cur, nxt = nxt, cur
            Hinv = cur[:, 6:12]

            # ---------------- JHt = (J @ Hinv) in Jt's [128, 8x6] layout:
            # dx then needs only ONE matmul per iteration
            # (dx^T = ones^T (JHt (.) r)), replacing the ycol/ysb/dxr chain
            JTsb = T([6, N], "JTsb")
            for c2 in range(NCH):
                tjp = psm.tile([6, 128], F32, tag="small", name="smallp")
                nc.tensor.transpose(tjp, Jt[:, 6 * c2:6 * c2 + 6], ident)
                nc.scalar.copy(JTsb[:, 128 * c2:128 * (c2 + 1)], tjp)
            pinvT = T([6, N], "pinvT")
            ppv = big_psum()
            nc.tensor.matmul(ppv[0:6, 0:512], Hinv, JTsb[:, 0:512])
            nc.tensor.matmul(ppv[0:6, 512:1024], Hinv, JTsb[:, 512:1024])
            nc.scalar.copy(pinvT[:, 0:512], ppv[0:6, 0:512])
            nc.scalar.copy(pinvT[:, 512:1024], ppv[0:6, 512:1024])
            JHt = T([128, 48], "JHt")
            for c2 in range(NCH):
                tjb = psm.tile([128, 6], F32, tag="small", name="smallp")
                nc.tensor.transpose(tjb, pinvT[:, 128 * c2:128 * (c2 + 1)],
                                    ident[0:6, 0:6])
                nc.scalar.copy(JHt[:, 6 * c2:6 * c2 + 6], tjb)

            # ---------------- GN iterations
            g_sb = T([4, 4], "g_sb")
            nc.vector.tensor_copy(g_sb, cI4)
            W1g = T([4, C1], "W1g")
            f = T([128, 8], "f_it")
            r = T([128, 8], "r_it")
            tmp48 = T([128, 48], "tmp48")
            s6 = T([128, 6], "s6")
            dxrow = T([1, 6], "dxrow")
            t2sb = T([1, 1], "t2sb")
            scr3 = T([1, 3], "scr3")
            sac3 = T([1, 3], "sac3")
            c3 = T([1, 3], "c3r")
            a13 = T([1, 3], "a13r")
            ww9 = T([1, 9], "ww9")
            W9 = T([1, 9], "W9r")
            R9 = T([1, 9], "R9r")
            V9 = T([1, 9], "V9r")
            prod9 = T([1, 9], "prod9")
            pv4 = T([1, 4], "pv4")
            Esb = T([4, 4], "Esb")
            ETsb = T([4, 4], "ETsb")
            V1sb = T([4, C1], "V1sb")
            cfill(pv4[:, 3:4], ones=True)

            def _rv(ap, off, dims):
                # free-dim strided/broadcast view of a [1, x] row AP
                return bass.AP(tensor=ap.tensor, offset=ap.offset + off,
                               ap=[list(ap.ap[0])] + dims)

            ww9v = ww9.rearrange("p (i j) -> p i j", j=3)
            prod9v = prod9.rearrange("p (i j) -> p i j", j=3)
            V9v = V9.rearrange("p (i j) -> p i j", j=3)
            JHtv_ci = JHt.rearrange("p (c i) -> p c i", i=6)
            tmpv_ci = tmp48.rearrange("p (c i) -> p c i", i=6)
            tmpv_ic = tmp48.rearrange("p (c i) -> p i c", i=6)

            for it in range(maxiter):
                if it > 0:
                    fwd12(W1g, p1hex, fast=False)
                    l3_chunks(m, None, fast=False)
                if it < maxiter - 1:
                    pe_fill(2)
                nc.vector.tensor_mul(f, m, a3)
                nc.vector.tensor_add(f, f, bb3)
                if it == maxiter - 1:
                    nc.vector.tensor_scalar_max(f, f, 0.0)
                    nc.vector.tensor_sub(r, f, f0)
                    break
                # r = relu(f) - f0 in one pass
                nc.vector.scalar_tensor_tensor(
                    out=r, in0=f, scalar=0.0, op0=ALU.max,
                    in1=f0, op1=ALU.subtract)
                # dx^T = ones^T (JHt (.) r): elementwise + col-sum + ONE mm
                rb = _bcast_free(r, 6)  # [p, 8, 0x6]
                nc.vector.tensor_tensor(out=tmpv_ci, in0=JHtv_ci, in1=rb,
                                        op=ALU.mult)
                nc.vector.tensor_reduce(s6, tmpv_ic, axis=AX.X, op=ALU.add)
                dxrp = psm.tile([1, 6], F32, tag="small", name="smallp")
                nc.tensor.matmul(dxrp, ones128, s6)
                nc.scalar.copy(dxrow, dxrp)
                pe_fill(5)
                # ---- row-form se(3) exp map: one all-DVE block in
                # partition 0, no cross-engine round-trips ----
                wv = dxrow[:, 0:3]
                nc.vector.tensor_tensor(out=scr3, in0=wv, in1=wv,
                                        op=ALU.mult)
                nc.vector.tensor_reduce(t2sb, scr3, axis=AX.X, op=ALU.add)
                # ww9[3i+j] = w_i * w_j
                nc.vector.tensor_tensor(
                    out=ww9v, in0=_rv(dxrow, 0, [[1, 3], [0, 3]]),
                    in1=_rv(dxrow, 0, [[0, 3], [1, 3]]), op=ALU.mult)
                # W9 = x*S0 + y*S1 + z*S2 (row-major skew)
                nc.vector.tensor_scalar_mul(W9, cS9r[:, 0:9],
                                            dxrow[:, 0:1])
                nc.vector.scalar_tensor_tensor(
                    out=W9, in0=cS9r[:, 9:18], scalar=dxrow[:, 1:2],
                    op0=ALU.mult, in1=W9, op1=ALU.add)
                nc.vector.scalar_tensor_tensor(
                    out=W9, in0=cS9r[:, 18:27], scalar=dxrow[:, 2:3],
                    op0=ALU.mult, in1=W9, op1=ALU.add)
                # series s1,s2,s3: Horner in t2, degree 3 (|t2|<~0.05)
                nc.vector.tensor_copy(sac3, cser[:, 9:12])
                for j in range(4, 7):
                    nc.vector.scalar_tensor_tensor(
                        out=sac3, in0=sac3, scalar=t2sb,
                        op0=ALU.mult, in1=cser[:, 3 * j:3 * j + 3],
                        op1=ALU.add)
                # a13 = 1 - t2*(s1,s2,s3)
                nc.vector.tensor_scalar_mul(c3, sac3, t2sb)
                nc.vector.scalar_tensor_tensor(
                    out=a13, in0=c3, scalar=-1.0, op0=ALU.mult,
                    in1=cones[0:1, 0:3], op1=ALU.add)
                # R = s1*W + (1-s2 t2)*I + s2*wwT ; V likewise with s2,s3
                nc.vector.tensor_scalar_mul(R9, W9, sac3[:, 0:1])
                nc.vector.scalar_tensor_tensor(
                    out=R9, in0=cI9r, scalar=a13[:, 1:2], op0=ALU.mult,
                    in1=R9, op1=ALU.add)
                nc.vector.scalar_tensor_tensor(
                    out=R9, in0=ww9, scalar=sac3[:, 1:2], op0=ALU.mult,
                    in1=R9, op1=ALU.add)
                nc.vector.tensor_scalar_mul(V9, W9, sac3[:, 1:2])
                nc.vector.scalar_tensor_tensor(
                    out=V9, in0=cI9r, scalar=a13[:, 2:3], op0=ALU.mult,
                    in1=V9, op1=ALU.add)
                nc.vector.scalar_tensor_tensor(
                    out=V9, in0=ww9, scalar=sac3[:, 2:3], op0=ALU.mult,
                    in1=V9, op1=ALU.add)
                # pv4[0:3] = V @ v
                nc.vector.tensor_tensor(
                    out=prod9v, in0=V9v,
                    in1=_rv(dxrow, 3, [[0, 3], [1, 3]]), op=ALU.mult)
                nc.vector.tensor_reduce(pv4[:, 0:3], prod9v, axis=AX.X,
                                        op=ALU.add)
                # E = [[R, p],[0,1]] assembled in PSUM: col-3 outer product
                # (full [4,4] coverage) + 3 row outer products e_k (x) R[k,:]
                etp = psm.tile([4, 4], F32, tag="small", name="smallp")
                nc.tensor.matmul(etp, pv4, cE4, start=True, stop=False)
                for k2 in range(3):
                    nc.tensor.matmul(etp[0:3, 0:3],
                                     cI9r[:, 3 * k2:3 * k2 + 3],
                                     R9[:, 3 * k2:3 * k2 + 3],
                                     start=False, stop=(k2 == 2))
                nc.scalar.copy(Esb, etp)
                # W1g for the next forward: W1g = g^T E^T W1aug, with the
                # state update deferred off the critical path
                v1p = psm.tile([4, C1], F32, tag="small", name="smallp")
                nc.tensor.matmul(v1p, Esb, W1aug)
                nc.scalar.copy(V1sb, v1p)
                wpp = psm.tile([4, C1], F32, tag="small", name="smallp")
                nc.tensor.matmul(wpp, g_sb, V1sb)
                nc.scalar.copy(W1g, wpp)
                # deferred state update g <- E g (runs during next forward)
                et2 = psm.tile([4, 4], F32, tag="small", name="smallp")
                nc.tensor.transpose(et2, Esb, ident[0:4, 0:4])
                nc.scalar.copy(ETsb, et2)
                g2p = psm.tile([4, 4], F32, tag="small", name="smallp")
                nc.tensor.matmul(g2p, ETsb, g_sb)
                nc.scalar.copy(g_sb, g2p)

            nc.sync.dma_start(
                out=outd.rearrange("(c p) -> p c", p=128), in_=r)

    _split_multi_waits(nc)
    return nc


# ---------------------------------------------------------------- host side
def _pack_params(inputs):
    # cols: [0:10] gamma(l1,l2,l3k*8), [10:20] beta, [20:30] rm, [30:40] rv,
    # [40:50] conv-bias; layer3 vectors in k-chunk layout
    prm = np.zeros((128, 50), np.float32)
    for g, grp in enumerate([("gamma1", "gamma2", "gamma3"),
                             ("beta1", "beta2", "beta3"),
                             ("rm1", "rm2", "rm3"),
                             ("rv1", "rv2", "rv3"),
                             ("b1", "b2", "b3")]):
        base = 10 * g
        prm[:C1, base + 0] = np.asarray(inputs[grp[0]], np.float32)
        prm[:C2, base + 1] = np.asarray(inputs[grp[1]], np.float32)
        v = np.asarray(inputs[grp[2]], np.float32).reshape(8, 128)
        prm[:, base + 2:base + 10] = v.T
    return prm


def make_in_maps(inputs):
    B = int(np.asarray(inputs["p0"]).shape[0])
    prm = _pack_params(inputs)
    W2T = np.ascontiguousarray(np.asarray(inputs["W2"], np.float32).T)
    W3n = np.ascontiguousarray(np.asarray(inputs["W3"], np.float32))
    W3T = np.ascontiguousarray(W3n.T)
    p0 = np.asarray(inputs["p0"], np.float32)
    p1 = np.asarray(inputs["p1"], np.float32)
    W1 = np.ascontiguousarray(np.asarray(inputs["W1"], np.float32))
    return [
        {
            "p0": np.ascontiguousarray(p0[b]),
            "p1": np.ascontiguousarray(p1[b]),
            "W1": W1,
            "W2T": W2T,
            "W3T": W3T,
            "W3": W3n,
            "prm": prm,
            "consts": CONSTS,
        }
        for b in range(B)
    ]


_NC_CACHE = {}
TRACE = False
LAST_RESULT = None
USE_F32R = False
REDUCE_TTR = True


def kernel(**inputs):
    global LAST_RESULT
    maxiter = int(np.asarray(inputs["maxiter"]))
    B = int(np.asarray(inputs["p0"]).shape[0])
    if maxiter <= 0:
        return np.zeros((B, K), np.float32)

    key = (maxiter, USE_F32R, REDUCE_TTR)
    if key not in _NC_CACHE:
        _NC_CACHE[key] = build_kernel(maxiter, use_f32r=USE_F32R,
                                      reduce_ttr=REDUCE_TTR)
    nc = _NC_CACHE[key]
    in_maps = make_in_maps(inputs)
    res = run_bass_kernel_spmd(nc, in_maps, core_ids=list(range(B)),
                               trace=TRACE)
    LAST_RESULT = res
    return np.stack([res.results[b]["out"] for b in range(B)], axis=0)



# revision 79
# speedup vs baseline: 1.2423x; 1.0868x over previous
"""AnalyticalPointNetLK forward on 8 Trainium2 NeuronCores.

Pure data parallel: batch element b -> core b. Everything (PointNet forward,
argmax, analytical Jacobian, 6x6 inverse, Gauss-Newton iterations, se(3) exp
map) runs on-device in one NEFF.

Self-contained: only needs the container's concourse stack.
"""
import sys
import types

import numpy as np


# ---------------------------------------------------------------- env setup
def _setup_env():
    try:
        import concourse.bass  # noqa: F401
    except ImportError:
        sys.path.insert(0, "/opt/trn_rl_repo")

    # Inject antenv.axon_hooks (missing in this image) so trace=True works.
    try:
        import antenv

        if not hasattr(antenv, "axon_hooks"):
            _m = types.ModuleType("antenv.axon_hooks")
            _m._hook = None
            _m.set_axon_ntff_profile_hook = lambda h: setattr(_m, "_hook", h)
            _m.get_axon_ntff_profile_hook = lambda: _m._hook
            sys.modules["antenv.axon_hooks"] = _m
            antenv.axon_hooks = _m
            try:
                from trn_agent_boot.trn_boot import _ntff_profile_via_ctypes

                h = _ntff_profile_via_ctypes("/opt/axon/libaxon_pjrt.so")
                if h is not None:
                    _m.set_axon_ntff_profile_hook(h)
            except Exception:
                pass
    except ImportError:
        pass

    # Split multi-wait exit Drain (this walrus rejects >1 sem wait on CTRL).
    from concourse.tile import TileContext
    from concourse.vector_clock import ScopedClock

    if not getattr(TileContext, "_drain_split_installed", False):

        def _patched(self, tick_clock, wait_clock):
            nc = self.nc
            drain_inst = nc.sync.drain()
            wait_clock.add_sem_waits(
                drain_inst.ins, ScopedClock({None: tick_clock.global_clock})
            )
            si = drain_inst.ins.sync_info
            if si is not None and si.on_wait and len(si.on_wait) > 1:
                waits = list(si.on_wait)
                si.on_wait = waits[:1]
                for w in waits[1:]:
                    extra = nc.sync.drain()
                    esi = extra.ins.sync_info
                    if esi is None:
                        import bass_rust

                        extra.ins.sync_info = bass_rust.SyncInfo(
                            on_wait=[w], on_update=[]
                        )
                    else:
                        esi.on_wait = [w]
            nc.all_engine_barrier()
            assert self.sems is not None
            popped = nc._tile_sem_poison_stack.pop()
            assert popped is self._sem_poison
            nc.clear_and_free_semaphores(list(self.sems.allocated().values()))
            nc.all_engine_barrier()

        TileContext._drain_and_barrier = _patched
        TileContext._drain_split_installed = True


_setup_env()

import concourse.bass as bass  # noqa: E402
import concourse.mybir as mybir  # noqa: E402
from concourse.tile import TileContext  # noqa: E402
from concourse.bass_utils import run_bass_kernel_spmd  # noqa: E402

F32 = mybir.dt.float32
F32R = mybir.dt.float32r
ALU = mybir.AluOpType
ACTF = mybir.ActivationFunctionType
AX = mybir.AxisListType

N, K, C1, C2 = 1024, 1024, 64, 128
NCH = 8
EPS = 1e-5
NEG_INF = -3.0e38

# exp-map series (Horner, highest degree first): s1=sin t/t, s2=(1-cos t)/t^2,
# s3=(t-sin t)/t^3 as series in t2=t^2, up to t^12
SER = np.array(
    [
        [1 / 6227020800, 1 / 87178291200, 1 / 1307674368000],
        [-1 / 39916800, -1 / 479001600, -1 / 6227020800],
        [1 / 362880, 1 / 3628800, 1 / 39916800],
        [-1 / 5040, -1 / 40320, -1 / 362880],
        [1 / 120, 1 / 720, 1 / 5040],
        [-1 / 6, -1 / 24, -1 / 120],
        [1.0, 0.5, 1 / 6],
    ],
    np.float32,
)

# ------------------------------------------------------------- consts blob
OFF_IDENT = 0
OFF_IOTA = OFF_IDENT + 128 * 128
OFF_SER = OFF_IOTA + 1024
OFF_S = OFF_SER + 21          # S0|S1|S2 skew generators, [3 rows x 9]
OFF_I3 = OFF_S + 27           # [3 x 3]
OFF_I34 = OFF_I3 + 9          # [3 x 4]
OFF_E4 = OFF_I34 + 12         # [1 x 4]
OFF_I4 = OFF_E4 + 4           # [4 x 4]
OFF_E16 = OFF_I4 + 16         # [16 x 3]
OFF_ONES = OFF_E16 + 48       # [6 x 8] ones
OFF_E6 = OFF_ONES + 48        # [6 x 36]: cE6[:, 6i:6i+6] = row-i-ones matrix
OFF_I6 = OFF_E6 + 216         # [6 x 6]
OFF_I9ROW = OFF_I6 + 36       # [1 x 9] I3 row-major
OFF_S9ROW = OFF_I9ROW + 9     # [1 x 27] skew generators row-major
OFF_ONE1K = OFF_S9ROW + 27    # [1024] ones
OFF_ZERO1K = OFF_ONE1K + 1024  # [1024] zeros
CONST_LEN = OFF_ZERO1K + 1024


def _build_consts():
    c = np.zeros(CONST_LEN, np.float32)
    c[OFF_IDENT:OFF_IDENT + 128 * 128] = np.eye(128, dtype=np.float32).ravel()
    c[OFF_IOTA:OFF_IOTA + 1024] = np.arange(1024, dtype=np.float32)
    c[OFF_SER:OFF_SER + 21] = SER.ravel()
    S0 = np.array([[0, 0, 0], [0, 0, -1], [0, 1, 0]], np.float32)
    S1 = np.array([[0, 0, 1], [0, 0, 0], [-1, 0, 0]], np.float32)
    S2 = np.array([[0, -1, 0], [1, 0, 0], [0, 0, 0]], np.float32)
    c[OFF_S:OFF_S + 27] = np.concatenate([S0, S1, S2], axis=1).ravel()
    c[OFF_I3:OFF_I3 + 9] = np.eye(3, dtype=np.float32).ravel()
    c[OFF_I34:OFF_I34 + 12] = np.eye(3, 4, dtype=np.float32).ravel()
    c[OFF_E4:OFF_E4 + 4] = np.array([0, 0, 0, 1], np.float32)
    c[OFF_I4:OFF_I4 + 16] = np.eye(4, dtype=np.float32).ravel()
    c[OFF_E16:OFF_E16 + 48] = np.eye(16, 3, dtype=np.float32).ravel()
    c[OFF_ONES:OFF_ONES + 48] = 1.0
    e6 = np.zeros((6, 36), np.float32)
    for i in range(6):
        e6[i, 6 * i:6 * i + 6] = 1.0
    c[OFF_E6:OFF_E6 + 216] = e6.ravel()
    c[OFF_I6:OFF_I6 + 36] = np.eye(6, dtype=np.float32).ravel()
    c[OFF_I9ROW:OFF_I9ROW + 9] = np.eye(3, dtype=np.float32).ravel()
    c[OFF_S9ROW:OFF_S9ROW + 27] = np.concatenate(
        [np.array([[0, 0, 0], [0, 0, -1], [0, 1, 0]], np.float32).ravel(),
         np.array([[0, 0, 1], [0, 0, 0], [-1, 0, 0]], np.float32).ravel(),
         np.array([[0, -1, 0], [1, 0, 0], [0, 0, 0]], np.float32).ravel()])
    c[OFF_ONE1K:OFF_ONE1K + 1024] = 1.0
    return c


CONSTS = _build_consts()


def _split_multi_waits(nc):
    """walrus (this build) accepts at most one sem wait per instruction on
    several opcode classes. Split any instruction with >1 waits by inserting
    same-engine nops, each carrying one wait, immediately before it."""
    import bass_rust

    def _make_nop(engine):
        h = nc.engines[engine]
        inst = h.nop(nofuse=True)
        # nop() appended to the current bb; detach it
        for f in nc.m.functions:
            for bb in f.blocks:
                lst = bb.instructions
                if lst and lst[-1] is inst.ins:
                    lst.pop()
                    return inst.ins
        raise RuntimeError("nop not found for detach")

    for f in nc.m.functions:
        for bb in f.blocks:
            lst = bb.instructions
            out = []
            changed = False
            for inst in list(lst):
                si = inst.sync_info
                if si is not None and si.on_wait and len(si.on_wait) > 1:
                    waits = list(si.on_wait)
                    for w in waits[:-1]:
                        nop = _make_nop(inst.engine)
                        nop.sync_info = bass_rust.SyncInfo(
                            on_wait=[w], on_update=[])
                        out.append(nop)
                    si.on_wait = [waits[-1]]
                    changed = True
                out.append(inst)
            if changed:
                lst.clear()
                lst.extend(out)


def _bcast_free(ap, count):
    """Append a stride-0 free dim (broadcast) to an AP view."""
    dims = [list(d) for d in ap.ap] + [[0, count]]
    return bass.AP(tensor=ap.tensor, offset=ap.offset, ap=dims)


def build_kernel(maxiter: int, use_f32r: bool = False, reduce_ttr: bool = True,
                 debug: bool = False):
    nc = bass.Bass("TRN2", target_bir_lowering=False, debug=False,
                   num_devices=8)

    def din(name, shape, dtype=F32):
        return nc.dram_tensor(name, shape, dtype, kind="ExternalInput").ap()

    p0d = din("p0", [N, 3])
    p1d = din("p1", [N, 3])
    W1d = din("W1", [C1, 3])
    W2Td = din("W2T", [C1, C2])
    W3Td = din("W3T", [C2, K])
    W3nd = din("W3", [K, C2])
    prmd = din("prm", [128, 50])
    constsd = din("consts", [CONST_LEN])
    outd = nc.dram_tensor("out", [K], F32, kind="ExternalOutput").ap()

    with TileContext(nc) as tc:
        with (
            tc.tile_pool(name="per", bufs=1) as per,
            tc.tile_pool(name="pbig", bufs=2, space="PSUM") as pbig,
            tc.tile_pool(name="psm", bufs=2, space="PSUM") as psm,
            tc.tile_pool(name="pfill", bufs=1, space="PSUM") as pfill,
            tc.tile_pool(name="bbp", bufs=2) as bbp,
            tc.tile_pool(name="dramp", bufs=1, space="DRAM") as dramp,
        ):
            def T(shape, tag, dtype=F32):
                return per.tile(shape, dtype, tag=tag, name=tag)

            def big_psum():
                return pbig.tile([128, 1024], F32, tag="big", name="bigp")

            def small_psum():
                return psm.tile([128, 64], F32, tag="small", name="smallp")

            def mmr(out, lhsT, rhs, fast=None, **kw):
                f = use_f32r if fast is None else (fast and use_f32r)
                if f:
                    nc.tensor.matmul(out, lhsT.bitcast(F32R),
                                     rhs.bitcast(F32R), **kw)
                else:
                    nc.tensor.matmul(out, lhsT, rhs, **kw)

            def rr(ap):
                # producers feeding an fp32r matmul must round to fp32r
                return ap.bitcast(F32R) if use_f32r else ap

            def cfill(dst, ones: bool, f32r: bool = False):
                # memset replacement (this walrus rejects InstMemset):
                # broadcast-DMA a constant vector from the consts blob
                p, fsz = dst.shape[0], 1
                for d in dst.shape[1:]:
                    fsz *= d
                assert fsz <= 1024
                off = OFF_ONE1K if ones else OFF_ZERO1K
                srcap = bass.AP(tensor=constsd.tensor,
                                offset=constsd.offset + off,
                                ap=[[0, p], [1, fsz]])
                if f32r and use_f32r:
                    srcap = srcap.bitcast(F32R)
                    dst = rr(dst)
                dmax(
                    out=dst.rearrange(
                        " ".join(f"d{i}" for i in range(len(dst.shape)))
                        + " -> d0 ("
                        + " ".join(f"d{i}" for i in range(1, len(dst.shape)))
                        + ")") if len(dst.shape) > 2 else dst,
                    in_=srcap)

            # ---------------- const + input DMAs
            # spread descriptor generation across four otherwise-idle
            # sequencers -- a single queue serializes ~25 input DMAs into
            # ~22us of setup
            _dmaq = [nc.sync, nc.gpsimd, nc.scalar]
            _dmai = [0]

            def dmax(out, in_):
                e = _dmaq[_dmai[0] % len(_dmaq)]
                _dmai[0] += 1
                e.dma_start(out=out, in_=in_)

            def cdma(shape, tag, off, length):
                t = T(shape, tag)
                dmax(out=t,
                     in_=constsd[off:off + length].rearrange(
                         "(p f) -> p f", p=shape[0]))
                return t

            # inputs needed by the weight-prep chain and first forward go
            # first; big late-use tensors (ident, W3T, W3n) go last
            prm = T([128, 50], "prm")
            dmax(out=prm, in_=prmd[:, :])
            W1sb = T([C1, 3], "W1sb")
            dmax(out=W1sb, in_=W1d[:, :])
            p0c4 = T([128, 32], "p0c")
            cfill(p0c4, ones=True)
            dmax(out=p0c4.rearrange("p (k d) -> p k d", d=4)[:, :, 0:3],
                 in_=p0d.rearrange("(p k) d -> p k d", p=128))
            W2T = T([C1, C2], "W2T")
            dmax(out=rr(W2T), in_=W2Td[:, :].bitcast(F32R)
                 if use_f32r else W2Td[:, :])
            p1c = T([128, 32], "p1c")
            cfill(p1c, ones=True)
            dmax(out=p1c.rearrange("p (k d) -> p k d", d=4)[:, :, 0:3],
                 in_=p1d.rearrange("(p k) d -> p k d", p=128))
            ident = cdma([128, 128], "ident", OFF_IDENT, 128 * 128)
            cser = cdma([1, 21], "cser", OFF_SER, 21)
            cE4 = cdma([1, 4], "cE4", OFF_E4, 4)
            cI4 = cdma([4, 4], "cI4", OFF_I4, 16)
            cones = cdma([6, 8], "cones", OFF_ONES, 48)
            cE6 = cdma([6, 36], "cE6", OFF_E6, 216)
            cI6 = cdma([6, 6], "cI6", OFF_I6, 36)
            cI9r = cdma([1, 9], "cI9r", OFF_I9ROW, 9)
            cS9r = cdma([1, 27], "cS9r", OFF_S9ROW, 27)
            W3T = T([C2, K], "W3T")
            dmax(out=rr(W3T), in_=W3Td[:, :].bitcast(F32R)
                 if use_f32r else W3Td[:, :])
            W3n = T([128, 1024], "W3n")
            w3n_src = bass.AP(tensor=W3nd.tensor, offset=W3nd.offset,
                              ap=[[128, 128], [16384, 8], [1, 128]])
            dmax(out=W3n.rearrange("p (c d) -> p c d", d=128), in_=w3n_src)
            ones128 = T([128, 1], "ones128")
            cfill(ones128, ones=True)

            # ---------------- param prep: a = gamma*rsqrt(rv+eps),
            # bb = a*(b-rm)+beta  (rsqrt: exact sqrt + bit-exact reciprocal)
            def bn_fold(gam, bet, rm, rv, bias, pshape, tagp):
                t = T(pshape, tagp + "_t")
                nc.vector.tensor_scalar_add(t, rv, EPS)
                s = T(pshape, tagp + "_s")
                nc.scalar.sqrt(s, t)
                y = T(pshape, tagp + "_y")
                nc.vector.reciprocal(y, s)
                a = T(pshape, tagp + "_a")
                nc.vector.tensor_mul(a, gam, y)
                bb = T(pshape, tagp + "_bb")
                nc.vector.tensor_sub(bb, bias, rm)
                nc.vector.tensor_mul(bb, bb, a)
                nc.vector.tensor_add(bb, bb, bet)
                return a, bb

            ablk, bbblk = bn_fold(prm[:, 0:10], prm[:, 10:20],
                                  prm[:, 20:30], prm[:, 30:40],
                                  prm[:, 40:50], [128, 10], "bn")
            a1, bb1 = ablk[0:C1, 0:1], bbblk[0:C1, 0:1]
            a2, bb2 = ablk[0:C2, 1:2], bbblk[0:C2, 1:2]
            a3, bb3 = ablk[:, 2:10], bbblk[:, 2:10]

            # ---------------- weight prep
            W1s = T([C1, 3], "W1s")
            nc.vector.tensor_scalar_mul(W1s, W1sb, a1)
            W1s4 = T([C1, 4], "W1s4")
            nc.vector.tensor_copy(W1s4[:, 0:3], W1s)
            nc.vector.tensor_copy(W1s4[:, 3:4], bb1)
            W1aug = T([4, C1], "W1aug")
            tp = small_psum()
            nc.tensor.transpose(tp[0:4, 0:C1], W1s4, ident[0:C1, 0:C1])
            nc.scalar.copy(rr(W1aug), tp[0:4, 0:C1])

            W2aug = T([C1 + 1, C2], "W2aug")
            a2dram = dramp.tile([C2], F32, tag="a2d", name="a2d")
            nc.sync.dma_start(out=a2dram, in_=a2)
            a2bc = T([C1, C2], "a2bc")
            a2bc_src = bass.AP(tensor=a2dram.tensor, offset=a2dram.offset,
                               ap=[[0, C1], [1, C2]])
            nc.sync.dma_start(out=a2bc, in_=a2bc_src)
            nc.vector.tensor_mul(rr(W2aug[0:C1, :]), W2T, a2bc)
            tp4 = psm.tile([128, 128], F32, tag="small", name="smallp")
            nc.tensor.transpose(tp4[0:1, 0:C2], bb2, ident[0:C2, 0:C2])
            nc.scalar.copy(rr(W2aug[C1:C1 + 1, :]), tp4[0:1, 0:C2])

            # ---------------- p0 / p1 transposed homogeneous
            p0hex = T([16, N], "p0hex")
            p1hex = T([4, N], "p1hex")
            cfill(p0hex, ones=False, f32r=True)
            for (srct, dsttile) in ((p0c4, p0hex), (p1c, p1hex)):
                tpp = big_psum()
                for c in range(8):
                    nc.tensor.transpose(
                        tpp[0:4, c * 128:(c + 1) * 128],
                        srct[:, c * 4:(c + 1) * 4], ident)
                nc.vector.tensor_copy(rr(dsttile[0:4, :]), tpp[0:4, 0:N])

            iotabc = T([128, 1024], "iotabc")
            iot_src = bass.AP(tensor=constsd.tensor,
                              offset=constsd.offset + OFF_IOTA,
                              ap=[[0, 128], [1, 1024]])
            dmax(out=iotabc, in_=iot_src)

            # ---------------- shared tiles
            scratch = T([128, 1024], "scratch")
            x1s = T([C1 + 1, N], "x1s")
            cfill(x1s[C1:C1 + 1, :], ones=True)
            x2s = T([C2, N], "x2s")

            # PE-warmth fillers: junk fp32 matmuls on constant tiles, issued
            # into per-iteration PE idle windows so the tensor clock stays
            # at its top p-state (cold restarts run ~1.8x slower)
            def pe_fill(n):
                if not USE_FILLERS:
                    return
                for _ in range(n):
                    fj = pfill.tile([128, 512], F32, tag="fill",
                                    name="fillp")
                    nc.tensor.matmul(fj, W3T[:, 0:128], W3n[:, 0:512])

            def fwd12(lhsT1, phex, fast=True):
                u1 = big_psum()
                nc.tensor.matmul(u1[0:C1, 0:512], lhsT1, phex[0:4, 0:512])
                nc.tensor.matmul(u1[0:C1, 512:1024], lhsT1,
                                 phex[0:4, 512:1024])
                nc.scalar.activation(x1s[0:C1, 0:512], u1[0:C1, 0:512],
                                     ACTF.Relu)
                nc.scalar.activation(x1s[0:C1, 512:1024],
                                     u1[0:C1, 512:1024], ACTF.Relu)
                pe_fill(1)  # bridge the relu latency, keep the clock hot
                u2 = big_psum()
                nc.tensor.matmul(u2[0:C2, 0:512], W2aug, x1s[:, 0:512])
                nc.tensor.matmul(u2[0:C2, 512:1024], W2aug,
                                 x1s[:, 512:1024])
                nc.scalar.activation(x2s[:, 0:512], u2[0:C2, 0:512],
                                     ACTF.Relu)
                nc.scalar.activation(x2s[:, 512:1024],
                                     u2[0:C2, 512:1024], ACTF.Relu)
                pe_fill(1)

            # chunks routed PSUM->SBUF via the idle Act engine: frees the
            # PSUM tile early and lets the DVE reduce run from SBUF (no
            # 120-cycle PSUM bubble, 2x port mode when available)
            # half-reduces free each chunk's PSUM tile ~0.8us earlier (the
            # WAR stall with bufs=2 otherwise paces the whole L3 phase);
            # one [128,16]->[128,8] combine at the end
            mtmp = T([128, 16], "mtmp")
            idxA = T([128, 8], "idxA")
            idxB = T([128, 8], "idxB")

            def l3_chunks(m_out, idx_out, fast=True, after_chunk=None):
                # single full-width reduce per chunk: measured equal cadence
                # to the half-split variant (DVE is the pacer either way)
                # with one fewer DVE op per chunk
                for c in range(NCH):
                    ps = big_psum()
                    lh = W3T[:, c * 128:(c + 1) * 128]
                    nc.tensor.matmul(ps[0:C2, 0:512], lh, x2s[:, 0:512])
                    nc.tensor.matmul(ps[0:C2, 512:1024], lh,
                                     x2s[:, 512:1024])
                    nc.vector.tensor_reduce(
                        m_out[:, c:c + 1], ps[:, 0:1024], axis=AX.X,
                        op=ALU.max)
                    if after_chunk is not None:
                        after_chunk(c)
                assert idx_out is None, "use l3_chunks_idx"

            def l3_chunks_idx(m_out, idx_out):
                for c in range(NCH):
                    ps = big_psum()
                    lh = W3T[:, c * 128:(c + 1) * 128]
                    nc.tensor.matmul(ps[0:C2, 0:512], lh, x2s[:, 0:512])
                    nc.tensor.matmul(ps[0:C2, 512:1024], lh,
                                     x2s[:, 512:1024])
                    nc.vector.tensor_reduce(
                        m_out[:, c:c + 1], ps[:, 0:1024], axis=AX.X,
                        op=ALU.max)
                    nc.vector.scalar_tensor_tensor(
                        out=scratch, in0=ps[:, 0:1024],
                        scalar=m_out[:, c:c + 1], in1=iotabc,
                        op0=ALU.is_ge, op1=ALU.mult,
                        accum_out=idx_out[:, c:c + 1])

            # ---------------- setup forward on p0 (masks + argmax)
            fwd12(W1aug, p0hex, fast=False)

            # masks via Act sign (x>=0 post-relu, so sign == is_gt-0 mask);
            # keeps the DVE free for the argmax phase
            m1f = T([C1, N], "m1f")
            nc.scalar.sign(m1f, x1s[0:C1, :])
            m2f = T([C2, N], "m2f")
            nc.scalar.sign(m2f, x2s)

            # ---------------- gather-table build (idx-independent, emitted
            # before the argmax L3 so its PE/DMA work overlaps DVE reduces)
            import concourse.tile as _tile_mod

            # m2f transposed: m2fT[:, 128*nu:...] = m2f chunk nu ^T
            m2fT = T([128, 1024], "m2fT")
            for nu in range(8):
                tpm = psm.tile([128, 128], F32, tag="small", name="smallp")
                nc.tensor.transpose(tpm, m2f[:, 128 * nu:128 * (nu + 1)],
                                    ident)
                nc.scalar.copy(m2fT[:, 128 * nu:128 * (nu + 1)], tpm)

            t1m = []
            for i in range(3):
                t = T([C1, N], f"t1m{i}")
                nc.scalar.mul(rr(t), m1f, W1s[:, i:i + 1])
                t1m.append(t)

            CATW = 388
            tcat = T([128, 8 * CATW], "tcat")
            tcat3 = tcat.rearrange("p (nu w) -> p nu w", w=CATW)

            # ---------------- argmax L3 on p0 first: the DVE is the pacer
            # in setup, and the gathers are gated by idxu32 anyway, so the
            # table-build DVE products go AFTER this stream
            m0 = T([128, 8], "m0")
            idxf = T([128, 8], "idxf")
            l3_chunks_idx(m0, idxf)

            f0 = T([128, 8], "f0")
            nc.vector.tensor_mul(f0, m0, a3)
            nc.vector.tensor_add(f0, f0, bb3)
            nc.vector.tensor_scalar_max(f0, f0, 0.0)
            dm3 = T([128, 8], "dm3")
            nc.vector.tensor_scalar(dm3, f0, 0.0, None, ALU.is_gt)
            nc.vector.tensor_mul(dm3, dm3, a3)
            idxu32 = T([128, 8], "idxu32", mybir.dt.uint32)
            nc.vector.tensor_copy(idxu32, idxf)

            # t2mT chunks: psum[128n, 512] holds nu-batch of 4 for one i
            for i in range(3):
                for half in range(2):
                    psb = pbig.tile([128, 512], F32, tag="big", name="ttp")
                    for q in range(4):
                        nu = half * 4 + q
                        mmr(psb[:, 128 * q:128 * (q + 1)],
                            t1m[i][:, 128 * nu:128 * (nu + 1)],
                            W2aug[0:C1, :])
                    nc.vector.tensor_tensor(
                        out=tcat3[:, 4 * half:4 * half + 4,
                                  128 * i:128 * (i + 1)],
                        in0=psb.rearrange("p (q d) -> p q d", d=128),
                        in1=m2fT.rearrange("p (nu d) -> p nu d", d=128)
                        [:, 4 * half:4 * half + 4, :],
                        op=ALU.mult)
            for nu in range(8):
                nc.scalar.copy(tcat3[:, nu, 384:388],
                               p0c4[:, 4 * nu:4 * nu + 4])

            tcatd = nc.dram_tensor("tcatd", [N, CATW], F32).ap()
            wr_inst = nc.sync.dma_start(
                out=tcatd.rearrange("(nu p) w -> p nu w", p=128), in_=tcat3)

            # ---------------- gathers (Pool desc-gen + DMA) overlapped with
            # the iteration-0 forward (PE/Act/DVE)
            gcs = []
            for c in range(NCH):
                gc = bbp.tile([128, CATW], F32, tag="gc", bufs=8, name="gc")
                gi = nc.gpsimd.indirect_dma_start(
                    out=gc[:, :], out_offset=None, in_=tcatd[:, :],
                    in_offset=bass.IndirectOffsetOnAxis(
                        ap=idxu32[:, c:c + 1], axis=0))
                _tile_mod.add_dep_helper(
                    gi.ins, wr_inst.ins, reason="gather waits table write")
                gcs.append(gc)

            # ---------------- iteration-0 forward with the Jacobian dot
            # products interleaved into the reduce stream (gather c is ready
            # by the time chunk c's reduce lands)
            fjgq = T([128, 24], "fjgq")
            pgs = T([128, 48], "pgs")

            def do_fjgq(c):
                gc = gcs[c]
                # prod[i, d] = gc[:, 128i+d] * W3n[:, 128c+d] then sum over d
                w3c = W3n[:, 128 * c:128 * (c + 1)]
                w3rep = bass.AP(tensor=w3c.tensor, offset=w3c.offset,
                                ap=[list(w3c.ap[0]), [0, 3],
                                    list(w3c.ap[1])])
                prodv = scratch[:, 0:384].rearrange(
                    "p (i d) -> p i d", d=128)
                nc.vector.tensor_tensor(
                    out=prodv,
                    in0=gc[:, 0:384].rearrange("p (i d) -> p i d", d=128),
                    in1=w3rep, op=ALU.mult)
                nc.vector.tensor_reduce(
                    fjgq[:, 3 * c:3 * c + 3], prodv, axis=AX.X, op=ALU.add)
                nc.scalar.copy(pgs[:, 6 * c:6 * c + 3], gc[:, 384:387])
                nc.scalar.copy(pgs[:, 6 * c + 3:6 * c + 6], gc[:, 384:387])

            m = T([128, 8], "m_it")
            fwd12(W1aug, p1hex, fast=False)
            l3_chunks(m, None, fast=False)
            pe_fill(6)
            for c in range(NCH):
                do_fjgq(c)

            # J assembly, vectorized across chunks with strided views
            fjgs = T([128, 48], "fjgs")
            Jt = T([128, 48], "Jt")
            scr48 = T([128, 48], "scr48")
            fjgqv = fjgq.rearrange("p (c i) -> p c i", i=3)
            fjgsv = fjgs.rearrange("p (c i) -> p c i", i=6)
            pgsv = pgs.rearrange("p (c i) -> p c i", i=6)
            Jtv = Jt.rearrange("p (c i) -> p c i", i=6)
            scr48v = scr48.rearrange("p (c i) -> p c i", i=6)
            dm3b = _bcast_free(dm3, 3)  # [p, 8, 0x3]
            nc.vector.tensor_tensor(out=fjgsv[:, :, 0:3], in0=fjgqv,
                                    in1=dm3b, op=ALU.mult)
            nc.vector.tensor_copy(fjgsv[:, :, 3:6], fjgsv[:, :, 0:3])
            # J[:,0:3] = (F2,F0,F1)*(Y,Z,X) - (F1,F2,F0)*(Z,X,Y)
            nc.vector.tensor_tensor(out=Jtv[:, :, 0:3],
                                    in0=fjgsv[:, :, 2:5],
                                    in1=pgsv[:, :, 1:4], op=ALU.mult)
            nc.vector.tensor_tensor(out=scr48v[:, :, 0:3],
                                    in0=fjgsv[:, :, 1:4],
                                    in1=pgsv[:, :, 2:5], op=ALU.mult)
            nc.vector.tensor_tensor(out=Jtv[:, :, 0:3], in0=Jtv[:, :, 0:3],
                                    in1=scr48v[:, :, 0:3], op=ALU.subtract)
            nc.vector.tensor_scalar_mul(Jtv[:, :, 3:6], fjgsv[:, :, 0:3],
                                        -1.0)
            Hp = psm.tile([6, 6], F32, tag="small", name="smallp")
            for c in range(NCH):
                nc.tensor.matmul(Hp, Jt[:, 6 * c:6 * c + 6],
                                 Jt[:, 6 * c:6 * c + 6],
                                 start=(c == 0), stop=(c == NCH - 1))

            # J^T chunk transposes only need Jt -- emit before the 6x6
            # inverse so the PE work overlaps the serial GJ chain
            JTsb = T([6, N], "JTsb")
            for c2 in range(NCH):
                tjp = psm.tile([6, 128], F32, tag="small", name="smallp")
                nc.tensor.transpose(tjp, Jt[:, 6 * c2:6 * c2 + 6], ident)
                nc.scalar.copy(JTsb[:, 128 * c2:128 * (c2 + 1)], tjp)

            # ---------------- 6x6 inverse (unpivoted Gauss-Jordan, SPD)
            M1 = T([6, 12], "M1")
            M2 = T([6, 12], "M2")
            rb6 = T([6, 1], "rb6")
            prow6 = T([6, 12], "prow6")
            ncol = T([6, 1], "ncol")
            nc.scalar.copy(M1[:, 0:6], Hp)
            nc.vector.tensor_copy(M1[:, 6:12], cI6)
            cur, nxt = M1, M2
            for i in range(6):
                pr6 = psm.tile([6, 12], F32, tag="small", name="smallp")
                nc.tensor.matmul(pr6, cE6[:, 6 * i:6 * i + 6], cur)
                nc.vector.reciprocal(rb6, pr6[:, i:i + 1])
                nc.vector.tensor_scalar_mul(prow6, pr6, rb6)
                nc.vector.tensor_scalar_mul(ncol, cur[:, i:i + 1], -1.0)
                nc.vector.tensor_add(ncol, ncol, cI6[:, i:i + 1])
                nc.vector.scalar_tensor_tensor(
                    out=nxt, in0=prow6, scalar=ncol, op0=ALU.mult,
                    in1=cur, op1=ALU.add)
                cur, nxt = nxt, cur
            Hinv = cur[:, 6:12]

            # ---------------- JHt = (J @ Hinv) in Jt's [128, 8x6] layout:
            # dx then needs only ONE matmul per iteration
            # (dx^T = ones^T (JHt (.) r)), replacing the ycol/ysb/dxr chain
            pinvT = T([6, N], "pinvT")
            ppv = big_psum()
            nc.tensor.matmul(ppv[0:6, 0:512], Hinv, JTsb[:, 0:512])
            nc.tensor.matmul(ppv[0:6, 512:1024], Hinv, JTsb[:, 512:1024])
            nc.scalar.copy(pinvT[:, 0:512], ppv[0:6, 0:512])
            nc.scalar.copy(pinvT[:, 512:1024], ppv[0:6, 512:1024])
            JHt = T([128, 48], "JHt")
            for c2 in range(NCH):
                tjb = psm.tile([128, 6], F32, tag="small", name="smallp")
                nc.tensor.transpose(tjb, pinvT[:, 128 * c2:128 * (c2 + 1)],
                                    ident[0:6, 0:6])
                nc.scalar.copy(JHt[:, 6 * c2:6 * c2 + 6], tjb)

            # ---------------- GN iterations
            g_sb = T([4, 4], "g_sb")
            nc.vector.tensor_copy(g_sb, cI4)
            W1g = T([4, C1], "W1g")
            f = T([128, 8], "f_it")
            r = T([128, 8], "r_it")
            tmp48 = T([128, 48], "tmp48")
            s6 = T([128, 6], "s6")
            dxrow = T([1, 6], "dxrow")
            t2sb = T([1, 1], "t2sb")
            scr3 = T([1, 3], "scr3")
            sac3 = T([1, 3], "sac3")
            c3 = T([1, 3], "c3r")
            a13 = T([1, 3], "a13r")
            ww9 = T([1, 9], "ww9")
            W9 = T([1, 9], "W9r")
            R9 = T([1, 9], "R9r")
            V9 = T([1, 9], "V9r")
            prod9 = T([1, 9], "prod9")
            pv4 = T([1, 4], "pv4")
            Esb = T([4, 4], "Esb")
            ETsb = T([4, 4], "ETsb")
            V1sb = T([4, C1], "V1sb")
            cfill(pv4[:, 3:4], ones=True)

            def _rv(ap, off, dims):
                # free-dim strided/broadcast view of a [1, x] row AP
                return bass.AP(tensor=ap.tensor, offset=ap.offset + off,
                               ap=[list(ap.ap[0])] + dims)

            ww9v = ww9.rearrange("p (i j) -> p i j", j=3)
            prod9v = prod9.rearrange("p (i j) -> p i j", j=3)
            V9v = V9.rearrange("p (i j) -> p i j", j=3)
            JHtv_ci = JHt.rearrange("p (c i) -> p c i", i=6)
            tmpv_ci = tmp48.rearrange("p (c i) -> p c i", i=6)
            tmpv_ic = tmp48.rearrange("p (c i) -> p i c", i=6)

            for it in range(maxiter):
                if it > 0:
                    fwd12(W1g, p1hex, fast=False)
                    # deferred state update g <- E g from the PREVIOUS
                    # iteration's tail: emitted after fwd12 so it cannot
                    # delay the L1 matmuls (it only must land before this
                    # iteration's wpp reads g_sb)
                    et2 = psm.tile([4, 4], F32, tag="small", name="smallp")
                    nc.tensor.transpose(et2, Esb, ident[0:4, 0:4])
                    nc.scalar.copy(ETsb, et2)
                    g2p = psm.tile([4, 4], F32, tag="small", name="smallp")
                    nc.tensor.matmul(g2p, ETsb, g_sb)
                    nc.scalar.copy(g_sb, g2p)
                    l3_chunks(m, None, fast=False)
                if it < maxiter - 1:
                    pe_fill(3)
                nc.vector.tensor_mul(f, m, a3)
                nc.vector.tensor_add(f, f, bb3)
                if it == maxiter - 1:
                    nc.vector.tensor_scalar_max(f, f, 0.0)
                    nc.vector.tensor_sub(r, f, f0)
                    break
                # r = relu(f) - f0 in one pass
                nc.vector.scalar_tensor_tensor(
                    out=r, in0=f, scalar=0.0, op0=ALU.max,
                    in1=f0, op1=ALU.subtract)
                # dx^T = ones^T (JHt (.) r): elementwise + col-sum + ONE mm
                rb = _bcast_free(r, 6)  # [p, 8, 0x6]
                nc.vector.tensor_tensor(out=tmpv_ci, in0=JHtv_ci, in1=rb,
                                        op=ALU.mult)
                nc.vector.tensor_reduce(s6, tmpv_ic, axis=AX.X, op=ALU.add)
                dxrp = psm.tile([1, 6], F32, tag="small", name="smallp")
                nc.tensor.matmul(dxrp, ones128, s6)
                # DVE copy: queue-local with the exp-map block (no Act hop)
                nc.vector.tensor_copy(dxrow, dxrp)
                pe_fill(6)
                # ---- row-form se(3) exp map: one all-DVE block in
                # partition 0, no cross-engine round-trips ----
                wv = dxrow[:, 0:3]
                nc.vector.tensor_tensor(out=scr3, in0=wv, in1=wv,
                                        op=ALU.mult)
                nc.vector.tensor_reduce(t2sb, scr3, axis=AX.X, op=ALU.add)
                # ww9[3i+j] = w_i * w_j
                nc.vector.tensor_tensor(
                    out=ww9v, in0=_rv(dxrow, 0, [[1, 3], [0, 3]]),
                    in1=_rv(dxrow, 0, [[0, 3], [1, 3]]), op=ALU.mult)
                # W9 = x*S0 + y*S1 + z*S2 (row-major skew)
                nc.vector.tensor_scalar_mul(W9, cS9r[:, 0:9],
                                            dxrow[:, 0:1])
                nc.vector.scalar_tensor_tensor(
                    out=W9, in0=cS9r[:, 9:18], scalar=dxrow[:, 1:2],
                    op0=ALU.mult, in1=W9, op1=ALU.add)
                nc.vector.scalar_tensor_tensor(
                    out=W9, in0=cS9r[:, 18:27], scalar=dxrow[:, 2:3],
                    op0=ALU.mult, in1=W9, op1=ALU.add)
                # series s1,s2,s3: Horner in t2, degree 3 (|t2|<~0.05)
                nc.vector.tensor_copy(sac3, cser[:, 9:12])
                for j in range(4, 7):
                    nc.vector.scalar_tensor_tensor(
                        out=sac3, in0=sac3, scalar=t2sb,
                        op0=ALU.mult, in1=cser[:, 3 * j:3 * j + 3],
                        op1=ALU.add)
                # a13 = 1 - t2*(s1,s2,s3)
                nc.vector.tensor_scalar_mul(c3, sac3, t2sb)
                nc.vector.scalar_tensor_tensor(
                    out=a13, in0=c3, scalar=-1.0, op0=ALU.mult,
                    in1=cones[0:1, 0:3], op1=ALU.add)
                # V first: pv4 unblocks the E-assembly's first matmul while
                # the DVE still computes R
                nc.vector.tensor_scalar_mul(V9, W9, sac3[:, 1:2])
                nc.vector.scalar_tensor_tensor(
                    out=V9, in0=cI9r, scalar=a13[:, 2:3], op0=ALU.mult,
                    in1=V9, op1=ALU.add)
                nc.vector.scalar_tensor_tensor(
                    out=V9, in0=ww9, scalar=sac3[:, 2:3], op0=ALU.mult,
                    in1=V9, op1=ALU.add)
                # pv4[0:3] = V @ v
                nc.vector.tensor_tensor(
                    out=prod9v, in0=V9v,
                    in1=_rv(dxrow, 3, [[0, 3], [1, 3]]), op=ALU.mult)
                nc.vector.tensor_reduce(pv4[:, 0:3], prod9v, axis=AX.X,
                                        op=ALU.add)
                # R = s1*W + (1-s2 t2)*I + s2*wwT
                nc.vector.tensor_scalar_mul(R9, W9, sac3[:, 0:1])
                nc.vector.scalar_tensor_tensor(
                    out=R9, in0=cI9r, scalar=a13[:, 1:2], op0=ALU.mult,
                    in1=R9, op1=ALU.add)
                nc.vector.scalar_tensor_tensor(
                    out=R9, in0=ww9, scalar=sac3[:, 1:2], op0=ALU.mult,
                    in1=R9, op1=ALU.add)
                # E = [[R, p],[0,1]] assembled in PSUM: col-3 outer product
                # (full [4,4] coverage) + 3 row outer products e_k (x) R[k,:]
                etp = psm.tile([4, 4], F32, tag="small", name="smallp")
                nc.tensor.matmul(etp, pv4, cE4, start=True, stop=False)
                for k2 in range(3):
                    nc.tensor.matmul(etp[0:3, 0:3],
                                     cI9r[:, 3 * k2:3 * k2 + 3],
                                     R9[:, 3 * k2:3 * k2 + 3],
                                     start=False, stop=(k2 == 2))
                nc.scalar.copy(Esb, etp)
                # W1g for the next forward: W1g = g^T E^T W1aug, with the
                # state update deferred off the critical path
                v1p = psm.tile([4, C1], F32, tag="small", name="smallp")
                nc.tensor.matmul(v1p, Esb, W1aug)
                nc.scalar.copy(V1sb, v1p)
                wpp = psm.tile([4, C1], F32, tag="small", name="smallp")
                nc.tensor.matmul(wpp, g_sb, V1sb)
                nc.scalar.copy(W1g, wpp)
                pe_fill(2)

            nc.sync.dma_start(
                out=outd.rearrange("(c p) -> p c", p=128), in_=r)

    _split_multi_waits(nc)
    return nc


# ---------------------------------------------------------------- host side
def _pack_params(inputs):
    # cols: [0:10] gamma(l1,l2,l3k*8), [10:20] beta, [20:30] rm, [30:40] rv,
    # [40:50] conv-bias; layer3 vectors in k-chunk layout
    prm = np.zeros((128, 50), np.float32)
    for g, grp in enumerate([("gamma1", "gamma2", "gamma3"),
                             ("beta1", "beta2", "beta3"),
                             ("rm1", "rm2", "rm3"),
                             ("rv1", "rv2", "rv3"),
                             ("b1", "b2", "b3")]):
        base = 10 * g
        prm[:C1, base + 0] = np.asarray(inputs[grp[0]], np.float32)
        prm[:C2, base + 1] = np.asarray(inputs[grp[1]], np.float32)
        v = np.asarray(inputs[grp[2]], np.float32).reshape(8, 128)
        prm[:, base + 2:base + 10] = v.T
    return prm


def make_in_maps(inputs):
    B = int(np.asarray(inputs["p0"]).shape[0])
    prm = _pack_params(inputs)
    W2T = np.ascontiguousarray(np.asarray(inputs["W2"], np.float32).T)
    W3n = np.ascontiguousarray(np.asarray(inputs["W3"], np.float32))
    W3T = np.ascontiguousarray(W3n.T)
    p0 = np.asarray(inputs["p0"], np.float32)
    p1 = np.asarray(inputs["p1"], np.float32)
    W1 = np.ascontiguousarray(np.asarray(inputs["W1"], np.float32))
    return [
        {
            "p0": np.ascontiguousarray(p0[b]),
            "p1": np.ascontiguousarray(p1[b]),
            "W1": W1,
            "W2T": W2T,
            "W3T": W3T,
            "W3": W3n,
            "prm": prm,
            "consts": CONSTS,
        }
        for b in range(B)
    ]


_NC_CACHE = {}
TRACE = False
LAST_RESULT = None
USE_F32R = False
REDUCE_TTR = True
USE_FILLERS = True


def kernel(**inputs):
    global LAST_RESULT
    maxiter = int(np.asarray(inputs["maxiter"]))
    B = int(np.asarray(inputs["p0"]).shape[0])
    if maxiter <= 0:
        return np.zeros((B, K), np.float32)

    key = (maxiter, USE_F32R, REDUCE_TTR)
    if key not in _NC_CACHE:
        _NC_CACHE[key] = build_kernel(maxiter, use_f32r=USE_F32R,
                                      reduce_ttr=REDUCE_TTR)
    nc = _NC_CACHE[key]
    in_maps = make_in_maps(inputs)
    res = run_bass_kernel_spmd(nc, in_maps, core_ids=list(range(B)),
                               trace=TRACE)
    LAST_RESULT = res
    return np.stack([res.results[b]["out"] for b in range(B)], axis=0)

